# revision 4
# baseline (speedup 1.0000x reference)
"""GATv2 (3-layer, 4-head) on 8 Trainium2 NeuronCores — Bass/Tile SPMD kernel.

Sharding: destination-node partition (graph parallel). Core c owns dst nodes
[c*NPC, (c+1)*NPC) in NBLK blocks of BLK. Edges (incl. mean-filled
self-loops) are bucketed by dst block; all cores run one shared SPMD
program over padded, per-core index data.

Compute dtype is bf16 (PSUM accumulation fp32). Per layer:
  1. sharded node matmuls xl = h@Wl (to DRAM for AllGather) and
     xr = h@Wr (kept in SBUF — dst rows are block-local)
  2. AllGather of the bf16 xl table (only collective)
  3. per dst-block: dma_gather xl[src] rows only; xr[dst] is assembled
     with a node-major one-hot indicator matmul from the SBUF xr block;
     z = ee + xl_g + xr_g accumulated in PSUM,
     leaky_relu via one scalar_tensor_tensor on DVE, att-dot + per-head
     reduce on DVE, exp on ScalarE (softmax max-shift dropped: alpha is
     shift-invariant, logits are O(1)),
     msg = w * xl_g via ScalarE activation-with-scale,
     unnormalized scatter out += A^T @ msg and denom += A^T @ w
     via edge-major one-hot indicator matmuls,
     then normalize via ScalarE activation-with-scale (folding 1/H),
     head-mean, bias, outer leaky_relu.
"""
import sys

sys.path.insert(0, "/opt/trn_rl_repo")
from contextlib import ExitStack

import numpy as np
import concourse.bacc as bacc
import concourse.mybir as mybir
import concourse.tile as tile
from concourse.bass_utils import run_bass_kernel_spmd
from concourse.library_config import mlp

f32 = mybir.dt.float32
bf16 = mybir.dt.bfloat16
i16 = mybir.dt.int16
ALU = mybir.AluOpType
AF = mybir.ActivationFunctionType
NP_BF16 = mybir.dt.np(bf16)

H = 4
D = 128
HD = H * D
F_IN = 128
NEG = 0.2
N_LAYERS = 3
C = 8
TILE = 128

# full-problem dims (overridable for small-scale sim tests)
DIMS = dict(N=20000, NPC=2500, BLK=125, NBLK=20)

_BUILD_CACHE = {}


# ----------------------------------------------------------------- host prep
def _pack_idxs(il):
    n = len(il)
    a = np.zeros((128, n // 16), np.int16)
    base = il.reshape(n // 16, 16).T
    for g in range(8):
        a[g * 16:(g + 1) * 16] = base
    return a


def _build_shards(edge_index, edge_attr, dims=DIMS):
    N, NPC, BLK, NBLK = dims["N"], dims["NPC"], dims["BLK"], dims["NBLK"]
    src = np.asarray(edge_index[0], np.int64)
    dst = np.asarray(edge_index[1], np.int64)
    ea = np.asarray(edge_attr, np.float32)

    ea_sum = np.zeros((N, 2), np.float32)
    np.add.at(ea_sum, dst, ea)
    cnt = np.bincount(dst, minlength=N).astype(np.float32)
    loop_attr = ea_sum / np.maximum(cnt, 1.0)[:, None]

    fsrc = np.concatenate([src, np.arange(N, dtype=np.int64)])
    fdst = np.concatenate([dst, np.arange(N, dtype=np.int64)])
    ffea = np.concatenate([ea, loop_attr], axis=0)

    key = fdst // NPC * NBLK + (fdst % NPC) // BLK
    order = np.argsort(key, kind="stable")
    kb = key[order]
    bounds = np.searchsorted(kb, np.arange(C * NBLK + 1))
    max_edges = int(np.max(np.diff(bounds)))
    tpb = (max_edges + TILE - 1) // TILE
    epb = tpb * TILE
    ec = NBLK * epb

    shards = []
    for c in range(C):
        s_src = np.zeros(ec, np.int16)
        s_dstloc = np.zeros(ec, np.int16)
        s_fea = np.zeros((ec, 2), np.float32)
        s_valid = np.zeros(ec, bool)
        for b in range(NBLK):
            k = c * NBLK + b
            el = order[bounds[k]:bounds[k + 1]]
            o = b * epb
            n = len(el)
            s_src[o:o + n] = fsrc[el].astype(np.int16)
            s_dstloc[o:o + n] = (fdst[el] - c * NPC).astype(np.int16)
            s_fea[o:o + n] = ffea[el]
            s_valid[o:o + n] = True
        t_ids = np.arange(ec) // TILE
        rel = s_dstloc.astype(np.float32) - (t_ids // tpb) * BLK
        rel[~s_valid] = -1.0  # padding matches no indicator column
        dstrel_f = np.ascontiguousarray(rel.reshape(ec // TILE, TILE).T)
        shards.append(dict(
            src_pk=_pack_idxs(s_src),
            feaT=np.ascontiguousarray(s_fea.T).astype(NP_BF16),
            dstrel=dstrel_f.astype(np.float32),
            # broadcast layout: every partition row = per-edge dst-rel value
            dstrelB=np.tile(rel.astype(NP_BF16)[None, :], (128, 1)),
        ))
    return shards, tpb


# --------------------------------------------------------------- device build
def _build(tpb, nzb, dims=DIMS, compile=True):
    key = (tpb, nzb, tuple(sorted(dims.items())))
    if key in _BUILD_CACHE:
        return _BUILD_CACHE[key]
    N, NPC, BLK, NBLK = dims["N"], dims["NPC"], dims["BLK"], dims["NBLK"]
    nz_bf, nz_bl, nz_br, nz_bo = nzb
    epb = tpb * TILE
    ec = NBLK * epb

    nc = bacc.Bacc("TRN2", target_bir_lowering=False, debug=False, num_devices=C)
    d_xT = nc.dram_tensor("xT", [F_IN, NPC], bf16, kind="ExternalInput")
    d_feaT = nc.dram_tensor("feaT", [2, ec], bf16, kind="ExternalInput")
    d_srcpk = nc.dram_tensor("src_pk", [128, ec // 16], i16, kind="ExternalInput")
    d_dstrel = nc.dram_tensor("dstrel", [128, ec // TILE], f32, kind="ExternalInput")
    d_dstrelB = nc.dram_tensor("dstrelB", [128, ec], bf16, kind="ExternalInput")
    d_eye = nc.dram_tensor("eye", [128, 128], bf16, kind="ExternalInput")
    d_iorow = nc.dram_tensor("iorow", [128, BLK], bf16, kind="ExternalInput")
    d_iocol = nc.dram_tensor("iocol", [BLK, 1], f32, kind="ExternalInput")
    d_Wf = nc.dram_tensor("Wf", [F_IN, D], bf16, kind="ExternalInput")
    d_Wl = nc.dram_tensor("Wl", [D, HD], bf16, kind="ExternalInput")
    d_Wr = nc.dram_tensor("Wr", [D, HD], bf16, kind="ExternalInput")
    d_We = nc.dram_tensor("We", [2, HD], bf16, kind="ExternalInput")
    d_attb = nc.dram_tensor("att_b", [128, HD], bf16, kind="ExternalInput")
    d_bf = nc.dram_tensor("bf_col", [128, 1], f32, kind="ExternalInput")
    d_blb = nc.dram_tensor("bl_b", [128, HD], f32, kind="ExternalInput")
    d_brb = nc.dram_tensor("br_b", [128, HD], f32, kind="ExternalInput")
    d_bob = nc.dram_tensor("bo_b", [128, D], f32, kind="ExternalInput")
    d_out = nc.dram_tensor("hout", [NPC, D], f32, kind="ExternalOutput")

    with tile.TileContext(nc) as tc, ExitStack() as ex:
        cst = ex.enter_context(tc.tile_pool(name="cst", bufs=1))
        dram = ex.enter_context(tc.tile_pool(name="dram", bufs=1, space="DRAM"))
        ps512 = ex.enter_context(tc.tile_pool(name="ps512", bufs=4, space="PSUM"))
        psO = ex.enter_context(tc.tile_pool(name="psO", bufs=2, space="PSUM"))
        psD = ex.enter_context(tc.tile_pool(name="psD", bufs=1, space="PSUM"))
        psT = ex.enter_context(tc.tile_pool(name="psT", bufs=1, space="PSUM"))
        gb1 = ex.enter_context(tc.tile_pool(name="gb1", bufs=2))
        scr = ex.enter_context(tc.tile_pool(name="scr", bufs=3))
        blkp = ex.enter_context(tc.tile_pool(name="blkp", bufs=2))
        evp = ex.enter_context(tc.tile_pool(name="evp", bufs=3))
        feap = ex.enter_context(tc.tile_pool(name="feap", bufs=2))

        nc.gpsimd.load_library(mlp)

        def ld(dt, shape, dtype=bf16):
            t = cst.tile(shape, dtype, name=f"sb_{dt.name}")
            nc.sync.dma_start(t[:], dt[:])
            return t

        eye = ld(d_eye, [128, 128])
        iorow = ld(d_iorow, [128, BLK])
        iocol = ld(d_iocol, [BLK, 1], f32)
        Wf = ld(d_Wf, [F_IN, D])
        Wl = ld(d_Wl, [D, HD])
        Wr = ld(d_Wr, [D, HD])
        We = ld(d_We, [2, HD])
        attb = ld(d_attb, [128, HD])
        xT = ld(d_xT, [F_IN, NPC])
        srcpk = ld(d_srcpk, [128, ec // 16], i16)
        dstrel = ld(d_dstrel, [128, ec // TILE], f32)
        dstrelB = ld(d_dstrelB, [128, ec])
        bf = ld(d_bf, [128, 1], f32) if nz_bf else None
        blb = ld(d_blb, [128, HD], f32) if nz_bl else None
        brb = ld(d_brb, [128, HD], f32) if nz_br else None
        bob = ld(d_bob, [128, D], f32) if nz_bo else None

        alpha_c = cst.tile([128, 1], f32, name="alpha_c")
        nc.vector.memset(alpha_c[:], NEG)
        hT = cst.tile([128, NPC], bf16, name="hT")
        xr_all = cst.tile([BLK, NBLK, HD], bf16, name="xr_all")
        agins = [dram.tile([NPC, HD], bf16, name=f"agin{i}")
                 for i in range(N_LAYERS)]
        agouts = [dram.tile([N, HD], bf16, addr_space="Shared", name=f"agout{i}")
                  for i in range(N_LAYERS)]

        # ---- layer-0 features, feature-major: h0T = Wf.T @ xT (+ bf)
        CH = min(NPC, 500)
        assert NPC % CH == 0
        for j in range(NPC // CH):
            ps = ps512.tile([128, CH], f32, tag="ps512")
            nc.tensor.matmul(ps[:], Wf[:], xT[:, j * CH:(j + 1) * CH],
                             start=True, stop=True)
            dst = hT[:, j * CH:(j + 1) * CH]
            if nz_bf:
                nc.vector.tensor_scalar_add(dst, ps[:], bf[:])
            else:
                nc.scalar.copy(dst, ps[:])

        for L in range(N_LAYERS):
            agin, agout = agins[L], agouts[L]
            # ---- node matmuls (own shard) -> xl to agin, xr to SBUF xr_all
            for m in range(NBLK):
                lh = hT[:, m * BLK:(m + 1) * BLK]
                psl = ps512.tile([BLK, HD], f32, tag="ps512")
                nc.tensor.matmul(psl[:], lh, Wl[:], start=True, stop=True)
                xle = evp.tile([BLK, HD], bf16, tag="ev")
                if nz_bl:
                    nc.vector.tensor_add(xle[:], psl[:], blb[:BLK, :])
                else:
                    nc.scalar.copy(xle[:], psl[:])
                nc.sync.dma_start(agin[m * BLK:(m + 1) * BLK, :], xle[:])
                psr = ps512.tile([BLK, HD], f32, tag="ps512")
                nc.tensor.matmul(psr[:], lh, Wr[:], start=True, stop=True)
                if nz_br:
                    nc.vector.tensor_add(xr_all[:, m, :], psr[:], brb[:BLK, :])
                else:
                    nc.scalar.copy(xr_all[:, m, :], psr[:])

            nc.gpsimd.collective_compute(
                "AllGather", ALU.bypass,
                replica_groups=[list(range(C))],
                ins=[agin.opt()], outs=[agout.opt()],
            )

            # ---- edge phase, per dst block
            for b in range(NBLK):
                e0 = b * epb
                GC = 4  # tiles per gather call (512 idxs: SWDGE ring limit)
                xlg = gb1.tile([128, tpb, HD], bf16, tag="xlg")
                for g0 in range(0, tpb, GC):
                    g1 = min(g0 + GC, tpb)
                    ne = (g1 - g0) * TILE
                    c0 = (e0 + g0 * TILE) // 16
                    nc.gpsimd.dma_gather(xlg[:, g0:g1, :], agout[:],
                                         srcpk[:, c0:c0 + ne // 16],
                                         ne, ne, HD)
                feaT = feap.tile([2, epb], bf16, tag="feaT")
                nc.sync.dma_start(feaT[:], d_feaT[:, e0:e0 + epb])
                # node-major indicator for the whole block: B[n,e]=1 iff dst(e)=n
                Bn = blkp.tile([BLK, tpb, TILE], bf16, tag="Bn")
                nc.vector.tensor_scalar(
                    Bn[:].rearrange("p a b -> p (a b)"),
                    dstrelB[:BLK, e0:e0 + epb], iocol[:], None, ALU.is_equal)
                lgb = blkp.tile([128, tpb, H], f32, tag="lgb")
                indb = blkp.tile([128, tpb, BLK], bf16, tag="indb")
                for t in range(tpb):
                    nc.vector.tensor_scalar(
                        indb[:, t, :], iorow[:],
                        dstrel[:, b * tpb + t:b * tpb + t + 1], None,
                        ALU.is_equal)
                    zp = ps512.tile([128, HD], f32, tag="ps512")
                    nc.tensor.matmul(zp[:], feaT[:, t * TILE:(t + 1) * TILE],
                                     We[:], start=True, stop=False)
                    nc.tensor.matmul(zp[:], Bn[:, t, :], xr_all[:, b, :],
                                     start=False, stop=False)
                    nc.tensor.matmul(zp[:], eye[:], xlg[:, t, :],
                                     start=False, stop=True)
                    lz = scr.tile([128, HD], bf16, tag="lz")
                    nc.scalar.activation(lz[:], zp[:], AF.Prelu,
                                         alpha=alpha_c[:])
                    y = scr.tile([128, HD], bf16, tag="y")
                    nc.vector.tensor_mul(y[:], lz[:], attb[:])
                    nc.vector.tensor_reduce(
                        lgb[:, t, :], y[:].rearrange("p (h d) -> p h d", h=H),
                        axis=mybir.AxisListType.X, op=ALU.add)
                webf = blkp.tile([128, tpb, H], f32, tag="webf")
                nc.scalar.activation(webf[:], lgb[:], AF.Exp)
                web = blkp.tile([128, tpb, H], bf16, tag="web")
                nc.scalar.activation(web[:], lgb[:], AF.Exp)
                outp = psO.tile([BLK, HD], f32, tag="psO")
                denp = psD.tile([BLK, H], f32, tag="psD")
                for t in range(tpb):
                    msg = scr.tile([128, HD], bf16, tag="msg")
                    for hh in range(H):
                        nc.gpsimd.tensor_scalar_mul(
                            msg[:, hh * D:(hh + 1) * D],
                            xlg[:, t, hh * D:(hh + 1) * D],
                            webf[:, t, hh:hh + 1])
                    nc.tensor.matmul(outp[:], indb[:, t, :], msg[:],
                                     start=(t == 0), stop=(t == tpb - 1))
                    nc.tensor.matmul(denp[:], indb[:, t, :], web[:, t, :],
                                     start=(t == 0), stop=(t == tpb - 1))
                invd = blkp.tile([BLK, H], f32, tag="invd")
                nc.vector.reciprocal(invd[:], denp[:])
                # fold the head-mean 1/H into the normalizer
                nc.vector.tensor_scalar_mul(invd[:], invd[:], 1.0 / H)
                o = blkp.tile([BLK, HD], bf16, tag="o")
                for hh in range(H):
                    nc.scalar.activation(
                        o[:, hh * D:(hh + 1) * D],
                        outp[:, hh * D:(hh + 1) * D],
                        AF.Copy, scale=invd[:, hh:hh + 1])
                s01 = blkp.tile([BLK, D], bf16, tag="s01")
                nc.vector.tensor_add(s01[:], o[:, 0:D], o[:, D:2 * D])
                s23 = blkp.tile([BLK, D], bf16, tag="s23")
                nc.vector.tensor_add(s23[:], o[:, 2 * D:3 * D], o[:, 3 * D:4 * D])
                sm = blkp.tile([BLK, D], bf16, tag="sm")
                if nz_bo:
                    nc.vector.tensor_add(sm[:], s01[:], s23[:])
                    nc.vector.tensor_add(sm[:], sm[:], bob[:BLK, :])
                else:
                    nc.vector.tensor_add(sm[:], s01[:], s23[:])
                if L == N_LAYERS - 1:
                    hb = blkp.tile([BLK, D], f32, tag="hbf")
                    nc.vector.scalar_tensor_tensor(
                        hb[:], sm[:], 0.01, sm[:], ALU.mult, ALU.max)
                    nc.sync.dma_start(d_out[b * BLK:(b + 1) * BLK, :], hb[:])
                else:
                    hb = blkp.tile([BLK, D], bf16, tag="hb")
                    nc.vector.scalar_tensor_tensor(
                        hb[:], sm[:], 0.01, sm[:], ALU.mult, ALU.max)
                    tp = psT.tile([128, BLK], bf16, tag="psT")
                    nc.tensor.transpose(tp[:], hb[:], eye[:BLK, :BLK])
                    nc.scalar.copy(hT[:, b * BLK:(b + 1) * BLK], tp[:])

    if compile:
        nc.compile()
    _BUILD_CACHE[key] = nc
    return nc


# ------------------------------------------------------------------ in_maps
def make_in_maps(inputs, dims=DIMS):
    N, NPC, BLK = dims["N"], dims["NPC"], dims["BLK"]
    x = np.asarray(inputs["x"], np.float32)
    Wf = np.ascontiguousarray(np.asarray(inputs["Wf"], np.float32))
    bf = np.asarray(inputs["bf"], np.float32)
    Wl = np.ascontiguousarray(np.asarray(inputs["Wl"], np.float32))
    bl = np.asarray(inputs["bl"], np.float32)
    Wr = np.ascontiguousarray(np.asarray(inputs["Wr"], np.float32))
    br = np.asarray(inputs["br"], np.float32)
    We = np.ascontiguousarray(np.asarray(inputs["We"], np.float32))
    att = np.asarray(inputs["att"], np.float32)
    bias_out = np.asarray(inputs["bias_out"], np.float32)

    shards, tpb = _build_shards(inputs["edge_index"], inputs["edge_attr"], dims)
    nzb = (bool(bf.any()), bool(bl.any()), bool(br.any()), bool(bias_out.any()))

    common = dict(
        eye=np.eye(128, dtype=NP_BF16),
        iorow=np.tile(np.arange(BLK, dtype=NP_BF16), (128, 1)),
        iocol=np.arange(BLK, dtype=np.float32).reshape(BLK, 1),
        Wf=Wf.astype(NP_BF16), Wl=Wl.astype(NP_BF16),
        Wr=Wr.astype(NP_BF16), We=We.astype(NP_BF16),
        att_b=np.tile(att.reshape(1, HD), (128, 1)).astype(NP_BF16),
        bf_col=np.ascontiguousarray(bf.reshape(D, 1)),
        bl_b=np.tile(bl.reshape(1, HD), (128, 1)).astype(np.float32),
        br_b=np.tile(br.reshape(1, HD), (128, 1)).astype(np.float32),
        bo_b=np.tile(bias_out.reshape(1, D), (128, 1)).astype(np.float32),
    )
    in_maps = []
    for c in range(C):
        sh = shards[c]
        m = dict(common)
        m["xT"] = np.ascontiguousarray(x[c * NPC:(c + 1) * NPC].T).astype(NP_BF16)
        m["feaT"] = sh["feaT"]
        m["src_pk"] = sh["src_pk"]
        m["dstrel"] = sh["dstrel"]
        m["dstrelB"] = sh["dstrelB"]
        in_maps.append(m)
    return in_maps, tpb, nzb


# -------------------------------------------------------------- bench hooks
def build_for_inputs(inputs):
    in_maps, tpb, nzb = make_in_maps(inputs, DIMS)
    nc = _build(tpb, nzb, DIMS)
    return nc, in_maps


def assemble_output(outs, out_names):
    NPC = DIMS["NPC"]
    got = np.asarray(outs[out_names.index("hout")]).reshape(C, NPC, -1)
    return got.reshape(C * NPC, -1).astype(np.float32)


# -------------------------------------------------------------------- kernel
def kernel(**inputs):
    in_maps, tpb, nzb = make_in_maps(inputs, DIMS)
    nc = _build(tpb, nzb, DIMS)
    res = run_bass_kernel_spmd(nc, in_maps, list(range(C)))
    NPC = DIMS["NPC"]
    return np.concatenate([res.results[c]["hout"] for c in range(C)], axis=0)


if __name__ == "__main__":
    nc = _build(10, (False, False, False, False), DIMS, compile=False)
    n_inst = sum(len(f.blocks[0].instructions) for f in nc.m.functions)
    print("trace-only build OK")


# revision 5
# speedup vs baseline: 3.1627x; 3.1627x over previous
"""GATv2 (3-layer, 4-head) on 8 Trainium2 NeuronCores — Bass/Tile SPMD kernel.

Sharding: destination-node partition (graph parallel). Core c owns dst nodes
[c*NPC, (c+1)*NPC) in NBLK blocks of BLK. Edges (incl. mean-filled
self-loops) are bucketed by dst block; all cores run one shared SPMD
program over padded, per-core index data.

Compute dtype is bf16 (PSUM accumulation fp32). Per layer:
  1. sharded node matmuls xl = h@Wl (to DRAM for AllGather) and
     xr = h@Wr (kept in SBUF — dst rows are block-local)
  2. AllGather of the bf16 xl table (only collective)
  3. per dst-block: dma_gather xl[src] rows only; xr[dst] is assembled
     with a node-major one-hot indicator matmul from the SBUF xr block;
     z = ee + xl_g + xr_g accumulated in PSUM,
     leaky_relu via one scalar_tensor_tensor on DVE, att-dot + per-head
     reduce on DVE, exp on ScalarE (softmax max-shift dropped: alpha is
     shift-invariant, logits are O(1)),
     msg = w * xl_g via ScalarE activation-with-scale,
     unnormalized scatter out += A^T @ msg and denom += A^T @ w
     via edge-major one-hot indicator matmuls,
     then normalize via ScalarE activation-with-scale (folding 1/H),
     head-mean, bias, outer leaky_relu.
"""
import sys

sys.path.insert(0, "/opt/trn_rl_repo")
from contextlib import ExitStack

import numpy as np
import concourse.bacc as bacc
import concourse.mybir as mybir
import concourse.tile as tile
from concourse.bass_utils import run_bass_kernel_spmd
from concourse.library_config import mlp

f32 = mybir.dt.float32
bf16 = mybir.dt.bfloat16
i16 = mybir.dt.int16
ALU = mybir.AluOpType
AF = mybir.ActivationFunctionType
NP_BF16 = mybir.dt.np(bf16)

H = 4
D = 128
HD = H * D
F_IN = 128
NEG = 0.2
N_LAYERS = 3
C = 8
TILE = 128

# full-problem dims (overridable for small-scale sim tests)
DIMS = dict(N=20000, NPC=2500, BLK=125, NBLK=20)

_BUILD_CACHE = {}


# ----------------------------------------------------------------- host prep
def _pack_idxs(il):
    n = len(il)
    a = np.zeros((128, n // 16), np.int16)
    base = il.reshape(n // 16, 16).T
    for g in range(8):
        a[g * 16:(g + 1) * 16] = base
    return a


def _build_shards(edge_index, edge_attr, dims=DIMS):
    N, NPC, BLK, NBLK = dims["N"], dims["NPC"], dims["BLK"], dims["NBLK"]
    src = np.asarray(edge_index[0], np.int64)
    dst = np.asarray(edge_index[1], np.int64)
    ea = np.asarray(edge_attr, np.float32)

    ea_sum = np.zeros((N, 2), np.float32)
    np.add.at(ea_sum, dst, ea)
    cnt = np.bincount(dst, minlength=N).astype(np.float32)
    loop_attr = ea_sum / np.maximum(cnt, 1.0)[:, None]

    fsrc = np.concatenate([src, np.arange(N, dtype=np.int64)])
    fdst = np.concatenate([dst, np.arange(N, dtype=np.int64)])
    ffea = np.concatenate([ea, loop_attr], axis=0)

    key = fdst // NPC * NBLK + (fdst % NPC) // BLK
    order = np.argsort(key, kind="stable")
    kb = key[order]
    bounds = np.searchsorted(kb, np.arange(C * NBLK + 1))
    max_edges = int(np.max(np.diff(bounds)))
    tpb = (max_edges + TILE - 1) // TILE
    epb = tpb * TILE
    ec = NBLK * epb

    shards = []
    for c in range(C):
        s_src = np.zeros(ec, np.int16)
        s_dstloc = np.zeros(ec, np.int16)
        s_fea = np.zeros((ec, 2), np.float32)
        s_valid = np.zeros(ec, bool)
        for b in range(NBLK):
            k = c * NBLK + b
            el = order[bounds[k]:bounds[k + 1]]
            o = b * epb
            n = len(el)
            s_src[o:o + n] = fsrc[el].astype(np.int16)
            s_dstloc[o:o + n] = (fdst[el] - c * NPC).astype(np.int16)
            s_fea[o:o + n] = ffea[el]
            s_valid[o:o + n] = True
        t_ids = np.arange(ec) // TILE
        rel = s_dstloc.astype(np.float32) - (t_ids // tpb) * BLK
        rel[~s_valid] = -1.0  # padding matches no indicator column
        dstrel_f = np.ascontiguousarray(rel.reshape(ec // TILE, TILE).T)
        shards.append(dict(
            src_pk=_pack_idxs(s_src),
            feaT=np.ascontiguousarray(s_fea.T).astype(NP_BF16),
            dstrel=dstrel_f.astype(np.float32),
            # broadcast layout: every partition row = per-edge dst-rel value
            dstrelB=np.tile(rel.astype(NP_BF16)[None, :], (128, 1)),
        ))
    return shards, tpb


# --------------------------------------------------------------- device build
def _build(tpb, nzb, dims=DIMS, compile=True):
    key = (tpb, nzb, tuple(sorted(dims.items())))
    if key in _BUILD_CACHE:
        return _BUILD_CACHE[key]
    N, NPC, BLK, NBLK = dims["N"], dims["NPC"], dims["BLK"], dims["NBLK"]
    nz_bf, nz_bl, nz_br, nz_bo = nzb
    epb = tpb * TILE
    ec = NBLK * epb

    nc = bacc.Bacc("TRN2", target_bir_lowering=False, debug=False, num_devices=C)
    d_xT = nc.dram_tensor("xT", [F_IN, NPC], bf16, kind="ExternalInput")
    d_feaT = nc.dram_tensor("feaT", [2, ec], bf16, kind="ExternalInput")
    d_srcpk = nc.dram_tensor("src_pk", [128, ec // 16], i16, kind="ExternalInput")
    d_dstrel = nc.dram_tensor("dstrel", [128, ec // TILE], f32, kind="ExternalInput")
    d_dstrelB = nc.dram_tensor("dstrelB", [128, ec], bf16, kind="ExternalInput")
    d_eye = nc.dram_tensor("eye", [128, 128], bf16, kind="ExternalInput")
    d_iorow = nc.dram_tensor("iorow", [128, BLK], bf16, kind="ExternalInput")
    d_iocol = nc.dram_tensor("iocol", [BLK, 1], f32, kind="ExternalInput")
    d_Wf = nc.dram_tensor("Wf", [F_IN, D], bf16, kind="ExternalInput")
    d_Wl = nc.dram_tensor("Wl", [D, HD], bf16, kind="ExternalInput")
    d_Wr = nc.dram_tensor("Wr", [D, HD], bf16, kind="ExternalInput")
    d_We = nc.dram_tensor("We", [2, HD], bf16, kind="ExternalInput")
    d_attb = nc.dram_tensor("att_b", [128, HD], bf16, kind="ExternalInput")
    d_bf = nc.dram_tensor("bf_col", [128, 1], f32, kind="ExternalInput")
    d_blb = nc.dram_tensor("bl_b", [128, HD], f32, kind="ExternalInput")
    d_brb = nc.dram_tensor("br_b", [128, HD], f32, kind="ExternalInput")
    d_bob = nc.dram_tensor("bo_b", [128, D], f32, kind="ExternalInput")
    d_out = nc.dram_tensor("hout", [NPC, D], f32, kind="ExternalOutput")

    with tile.TileContext(nc) as tc, ExitStack() as ex:
        cst = ex.enter_context(tc.tile_pool(name="cst", bufs=1))
        dram = ex.enter_context(tc.tile_pool(name="dram", bufs=1, space="DRAM"))
        ps512 = ex.enter_context(tc.tile_pool(name="ps512", bufs=4, space="PSUM"))
        psO = ex.enter_context(tc.tile_pool(name="psO", bufs=2, space="PSUM"))
        psD = ex.enter_context(tc.tile_pool(name="psD", bufs=1, space="PSUM"))
        psT = ex.enter_context(tc.tile_pool(name="psT", bufs=1, space="PSUM"))
        gb1 = ex.enter_context(tc.tile_pool(name="gb1", bufs=2))
        scr = ex.enter_context(tc.tile_pool(name="scr", bufs=3))
        blkp = ex.enter_context(tc.tile_pool(name="blkp", bufs=2))
        evp = ex.enter_context(tc.tile_pool(name="evp", bufs=3))
        feap = ex.enter_context(tc.tile_pool(name="feap", bufs=2))

        nc.gpsimd.load_library(mlp)

        def ld(dt, shape, dtype=bf16):
            t = cst.tile(shape, dtype, name=f"sb_{dt.name}")
            nc.sync.dma_start(t[:], dt[:])
            return t

        eye = ld(d_eye, [128, 128])
        iorow = ld(d_iorow, [128, BLK])
        iocol = ld(d_iocol, [BLK, 1], f32)
        Wf = ld(d_Wf, [F_IN, D])
        Wl = ld(d_Wl, [D, HD])
        Wr = ld(d_Wr, [D, HD])
        We = ld(d_We, [2, HD])
        attb = ld(d_attb, [128, HD])
        xT = ld(d_xT, [F_IN, NPC])
        srcpk = ld(d_srcpk, [128, ec // 16], i16)
        dstrel = ld(d_dstrel, [128, ec // TILE], f32)
        dstrelB = ld(d_dstrelB, [128, ec])
        bf = ld(d_bf, [128, 1], f32) if nz_bf else None
        blb = ld(d_blb, [128, HD], f32) if nz_bl else None
        brb = ld(d_brb, [128, HD], f32) if nz_br else None
        bob = ld(d_bob, [128, D], f32) if nz_bo else None

        alpha_c = cst.tile([128, 1], f32, name="alpha_c")
        nc.vector.memset(alpha_c[:], NEG)
        hT = cst.tile([128, NPC], bf16, name="hT")
        xr_all = cst.tile([BLK, NBLK, HD], bf16, name="xr_all")
        agins = [dram.tile([NPC, HD], bf16, name=f"agin{i}")
                 for i in range(N_LAYERS)]
        agouts = [dram.tile([N, HD], bf16, addr_space="Shared", name=f"agout{i}")
                  for i in range(N_LAYERS)]

        # ---- layer-0 features, feature-major: h0T = Wf.T @ xT (+ bf)
        CH = min(NPC, 500)
        assert NPC % CH == 0
        for j in range(NPC // CH):
            ps = ps512.tile([128, CH], f32, tag="ps512")
            nc.tensor.matmul(ps[:], Wf[:], xT[:, j * CH:(j + 1) * CH],
                             start=True, stop=True)
            dst = hT[:, j * CH:(j + 1) * CH]
            if nz_bf:
                nc.vector.tensor_scalar_add(dst, ps[:], bf[:])
            else:
                nc.scalar.copy(dst, ps[:])

        for L in range(N_LAYERS):
            agin, agout = agins[L], agouts[L]
            # ---- node matmuls (own shard) -> xl to agin, xr to SBUF xr_all
            for m in range(NBLK):
                lh = hT[:, m * BLK:(m + 1) * BLK]
                psl = ps512.tile([BLK, HD], f32, tag="ps512")
                nc.tensor.matmul(psl[:], lh, Wl[:], start=True, stop=True)
                xle = evp.tile([BLK, HD], bf16, tag="ev")
                if nz_bl:
                    nc.vector.tensor_add(xle[:], psl[:], blb[:BLK, :])
                else:
                    nc.scalar.copy(xle[:], psl[:])
                nc.sync.dma_start(agin[m * BLK:(m + 1) * BLK, :], xle[:])
                psr = ps512.tile([BLK, HD], f32, tag="ps512")
                nc.tensor.matmul(psr[:], lh, Wr[:], start=True, stop=True)
                if nz_br:
                    nc.vector.tensor_add(xr_all[:, m, :], psr[:], brb[:BLK, :])
                else:
                    nc.scalar.copy(xr_all[:, m, :], psr[:])

            nc.gpsimd.collective_compute(
                "AllGather", ALU.bypass,
                replica_groups=[list(range(C))],
                ins=[agin.opt()], outs=[agout.opt()],
            )

            # ---- edge phase, per dst block
            for b in range(NBLK):
                e0 = b * epb
                GC = 4  # tiles per gather call (512 idxs: SWDGE ring limit)
                xlg = gb1.tile([128, tpb, HD], bf16, tag="xlg")
                for g0 in range(0, tpb, GC):
                    g1 = min(g0 + GC, tpb)
                    ne = (g1 - g0) * TILE
                    c0 = (e0 + g0 * TILE) // 16
                    nc.gpsimd.dma_gather(xlg[:, g0:g1, :], agout[:],
                                         srcpk[:, c0:c0 + ne // 16],
                                         ne, ne, HD)
                feaT = feap.tile([2, epb], bf16, tag="feaT")
                nc.sync.dma_start(feaT[:], d_feaT[:, e0:e0 + epb])
                # node-major indicator for the whole block: B[n,e]=1 iff dst(e)=n
                Bn = blkp.tile([BLK, tpb, TILE], bf16, tag="Bn")
                nc.vector.tensor_scalar(
                    Bn[:].rearrange("p a b -> p (a b)"),
                    dstrelB[:BLK, e0:e0 + epb], iocol[:], None, ALU.is_equal)
                lgb = blkp.tile([128, tpb, H], f32, tag="lgb")
                indb = blkp.tile([128, tpb, BLK], bf16, tag="indb")
                for t in range(tpb):
                    nc.vector.tensor_scalar(
                        indb[:, t, :], iorow[:],
                        dstrel[:, b * tpb + t:b * tpb + t + 1], None,
                        ALU.is_equal)
                    zp = ps512.tile([128, HD], f32, tag="ps512")
                    nc.tensor.matmul(zp[:], feaT[:, t * TILE:(t + 1) * TILE],
                                     We[:], start=True, stop=False)
                    nc.tensor.matmul(zp[:], Bn[:, t, :], xr_all[:, b, :],
                                     start=False, stop=False)
                    nc.tensor.matmul(zp[:], eye[:], xlg[:, t, :],
                                     start=False, stop=True)
                    lz = scr.tile([128, HD], bf16, tag="lz")
                    nc.scalar.activation(lz[:], zp[:], AF.Prelu,
                                         alpha=alpha_c[:])
                    y = scr.tile([128, HD], bf16, tag="y")
                    nc.vector.tensor_mul(y[:], lz[:], attb[:])
                    nc.vector.tensor_reduce(
                        lgb[:, t, :], y[:].rearrange("p (h d) -> p h d", h=H),
                        axis=mybir.AxisListType.X, op=ALU.add)
                webf = blkp.tile([128, tpb, H], f32, tag="webf")
                nc.scalar.activation(webf[:], lgb[:], AF.Exp)
                web = blkp.tile([128, tpb, H], bf16, tag="web")
                nc.scalar.activation(web[:], lgb[:], AF.Exp)
                outp = psO.tile([BLK, HD], f32, tag="psO")
                denp = psD.tile([BLK, H], f32, tag="psD")
                for t in range(tpb):
                    msg = scr.tile([128, HD], bf16, tag="msg")
                    for hh in range(H):
                        nc.scalar.activation(
                            msg[:, hh * D:(hh + 1) * D],
                            xlg[:, t, hh * D:(hh + 1) * D],
                            AF.Copy, scale=webf[:, t, hh:hh + 1])
                    nc.tensor.matmul(outp[:], indb[:, t, :], msg[:],
                                     start=(t == 0), stop=(t == tpb - 1))
                    nc.tensor.matmul(denp[:], indb[:, t, :], web[:, t, :],
                                     start=(t == 0), stop=(t == tpb - 1))
                invd = blkp.tile([BLK, H], f32, tag="invd")
                nc.vector.reciprocal(invd[:], denp[:])
                # fold the head-mean 1/H into the normalizer
                nc.vector.tensor_scalar_mul(invd[:], invd[:], 1.0 / H)
                o = blkp.tile([BLK, HD], bf16, tag="o")
                for hh in range(H):
                    nc.scalar.activation(
                        o[:, hh * D:(hh + 1) * D],
                        outp[:, hh * D:(hh + 1) * D],
                        AF.Copy, scale=invd[:, hh:hh + 1])
                s01 = blkp.tile([BLK, D], bf16, tag="s01")
                nc.vector.tensor_add(s01[:], o[:, 0:D], o[:, D:2 * D])
                s23 = blkp.tile([BLK, D], bf16, tag="s23")
                nc.vector.tensor_add(s23[:], o[:, 2 * D:3 * D], o[:, 3 * D:4 * D])
                sm = blkp.tile([BLK, D], bf16, tag="sm")
                if nz_bo:
                    nc.vector.tensor_add(sm[:], s01[:], s23[:])
                    nc.vector.tensor_add(sm[:], sm[:], bob[:BLK, :])
                else:
                    nc.vector.tensor_add(sm[:], s01[:], s23[:])
                if L == N_LAYERS - 1:
                    hb = blkp.tile([BLK, D], f32, tag="hbf")
                    nc.vector.scalar_tensor_tensor(
                        hb[:], sm[:], 0.01, sm[:], ALU.mult, ALU.max)
                    nc.sync.dma_start(d_out[b * BLK:(b + 1) * BLK, :], hb[:])
                else:
                    hb = blkp.tile([BLK, D], bf16, tag="hb")
                    nc.vector.scalar_tensor_tensor(
                        hb[:], sm[:], 0.01, sm[:], ALU.mult, ALU.max)
                    tp = psT.tile([128, BLK], bf16, tag="psT")
                    nc.tensor.transpose(tp[:], hb[:], eye[:BLK, :BLK])
                    nc.scalar.copy(hT[:, b * BLK:(b + 1) * BLK], tp[:])

    if compile:
        nc.compile()
    _BUILD_CACHE[key] = nc
    return nc


# ------------------------------------------------------------------ in_maps
def make_in_maps(inputs, dims=DIMS):
    N, NPC, BLK = dims["N"], dims["NPC"], dims["BLK"]
    x = np.asarray(inputs["x"], np.float32)
    Wf = np.ascontiguousarray(np.asarray(inputs["Wf"], np.float32))
    bf = np.asarray(inputs["bf"], np.float32)
    Wl = np.ascontiguousarray(np.asarray(inputs["Wl"], np.float32))
    bl = np.asarray(inputs["bl"], np.float32)
    Wr = np.ascontiguousarray(np.asarray(inputs["Wr"], np.float32))
    br = np.asarray(inputs["br"], np.float32)
    We = np.ascontiguousarray(np.asarray(inputs["We"], np.float32))
    att = np.asarray(inputs["att"], np.float32)
    bias_out = np.asarray(inputs["bias_out"], np.float32)

    shards, tpb = _build_shards(inputs["edge_index"], inputs["edge_attr"], dims)
    nzb = (bool(bf.any()), bool(bl.any()), bool(br.any()), bool(bias_out.any()))

    common = dict(
        eye=np.eye(128, dtype=NP_BF16),
        iorow=np.tile(np.arange(BLK, dtype=NP_BF16), (128, 1)),
        iocol=np.arange(BLK, dtype=np.float32).reshape(BLK, 1),
        Wf=Wf.astype(NP_BF16), Wl=Wl.astype(NP_BF16),
        Wr=Wr.astype(NP_BF16), We=We.astype(NP_BF16),
        att_b=np.tile(att.reshape(1, HD), (128, 1)).astype(NP_BF16),
        bf_col=np.ascontiguousarray(bf.reshape(D, 1)),
        bl_b=np.tile(bl.reshape(1, HD), (128, 1)).astype(np.float32),
        br_b=np.tile(br.reshape(1, HD), (128, 1)).astype(np.float32),
        bo_b=np.tile(bias_out.reshape(1, D), (128, 1)).astype(np.float32),
    )
    in_maps = []
    for c in range(C):
        sh = shards[c]
        m = dict(common)
        m["xT"] = np.ascontiguousarray(x[c * NPC:(c + 1) * NPC].T).astype(NP_BF16)
        m["feaT"] = sh["feaT"]
        m["src_pk"] = sh["src_pk"]
        m["dstrel"] = sh["dstrel"]
        m["dstrelB"] = sh["dstrelB"]
        in_maps.append(m)
    return in_maps, tpb, nzb


# -------------------------------------------------------------- bench hooks
def build_for_inputs(inputs):
    in_maps, tpb, nzb = make_in_maps(inputs, DIMS)
    nc = _build(tpb, nzb, DIMS)
    return nc, in_maps


def assemble_output(outs, out_names):
    NPC = DIMS["NPC"]
    got = np.asarray(outs[out_names.index("hout")]).reshape(C, NPC, -1)
    return got.reshape(C * NPC, -1).astype(np.float32)


# -------------------------------------------------------------------- kernel
def kernel(**inputs):
    in_maps, tpb, nzb = make_in_maps(inputs, DIMS)
    nc = _build(tpb, nzb, DIMS)
    res = run_bass_kernel_spmd(nc, in_maps, list(range(C)))
    NPC = DIMS["NPC"]
    return np.concatenate([res.results[c]["hout"] for c in range(C)], axis=0)


if __name__ == "__main__":
    nc = _build(10, (False, False, False, False), DIMS, compile=False)
    n_inst = sum(len(f.blocks[0].instructions) for f in nc.m.functions)
    print("trace-only build OK")


# revision 8
# speedup vs baseline: 3.4398x; 1.0876x over previous
"""GATv2 (3-layer, 4-head) on 8 Trainium2 NeuronCores — Bass/Tile SPMD kernel.

Sharding: destination-node partition (graph parallel). Core c owns dst nodes
[c*NPC, (c+1)*NPC) in NBLK blocks of BLK. Edges (incl. mean-filled
self-loops) are bucketed by dst block; all cores run one shared SPMD
program over padded, per-core index data.

Compute dtype is bf16 (PSUM accumulation fp32). Per layer:
  1. sharded node matmuls xl = h@Wl (to DRAM for AllGather) and
     xr = h@Wr (kept in SBUF — dst rows are block-local)
  2. AllGather of the bf16 xl table (only collective)
  3. per dst-block: dma_gather xl[src] rows only; xr[dst] is assembled
     with a node-major one-hot indicator matmul from the SBUF xr block;
     z = ee + xl_g + xr_g accumulated in PSUM,
     leaky_relu via one scalar_tensor_tensor on DVE, att-dot + per-head
     reduce on DVE, exp on ScalarE (softmax max-shift dropped: alpha is
     shift-invariant, logits are O(1)),
     msg = w * xl_g via ScalarE activation-with-scale,
     unnormalized scatter out += A^T @ msg and denom += A^T @ w
     via edge-major one-hot indicator matmuls,
     then normalize via ScalarE activation-with-scale (folding 1/H),
     head-mean, bias, outer leaky_relu.
"""
import sys

sys.path.insert(0, "/opt/trn_rl_repo")
from contextlib import ExitStack

import numpy as np
import concourse.bacc as bacc
import concourse.mybir as mybir
import concourse.tile as tile
from concourse.bass_utils import run_bass_kernel_spmd
from concourse.library_config import mlp

f32 = mybir.dt.float32
bf16 = mybir.dt.bfloat16
i16 = mybir.dt.int16
ALU = mybir.AluOpType
AF = mybir.ActivationFunctionType
NP_BF16 = mybir.dt.np(bf16)

H = 4
D = 128
HD = H * D
F_IN = 128
NEG = 0.2
N_LAYERS = 3
C = 8
TILE = 128

# full-problem dims (overridable for small-scale sim tests)
DIMS = dict(N=20000, NPC=2500, BLK=125, NBLK=20)

_BUILD_CACHE = {}


# ----------------------------------------------------------------- host prep
def _pack_idxs(il):
    n = len(il)
    a = np.zeros((128, n // 16), np.int16)
    base = il.reshape(n // 16, 16).T
    for g in range(8):
        a[g * 16:(g + 1) * 16] = base
    return a


def _build_shards(edge_index, edge_attr, dims=DIMS):
    N, NPC, BLK, NBLK = dims["N"], dims["NPC"], dims["BLK"], dims["NBLK"]
    src = np.asarray(edge_index[0], np.int64)
    dst = np.asarray(edge_index[1], np.int64)
    ea = np.asarray(edge_attr, np.float32)

    ea_sum = np.zeros((N, 2), np.float32)
    np.add.at(ea_sum, dst, ea)
    cnt = np.bincount(dst, minlength=N).astype(np.float32)
    loop_attr = ea_sum / np.maximum(cnt, 1.0)[:, None]

    fsrc = np.concatenate([src, np.arange(N, dtype=np.int64)])
    fdst = np.concatenate([dst, np.arange(N, dtype=np.int64)])
    ffea = np.concatenate([ea, loop_attr], axis=0)

    key = fdst // NPC * NBLK + (fdst % NPC) // BLK
    order = np.argsort(key, kind="stable")
    kb = key[order]
    bounds = np.searchsorted(kb, np.arange(C * NBLK + 1))
    max_edges = int(np.max(np.diff(bounds)))
    tpb = (max_edges + TILE - 1) // TILE
    epb = tpb * TILE
    ec = NBLK * epb

    shards = []
    for c in range(C):
        s_src = np.zeros(ec, np.int16)
        s_dstloc = np.zeros(ec, np.int16)
        s_fea = np.zeros((ec, 2), np.float32)
        s_valid = np.zeros(ec, bool)
        for b in range(NBLK):
            k = c * NBLK + b
            el = order[bounds[k]:bounds[k + 1]]
            o = b * epb
            n = len(el)
            s_src[o:o + n] = fsrc[el].astype(np.int16)
            s_dstloc[o:o + n] = (fdst[el] - c * NPC).astype(np.int16)
            s_fea[o:o + n] = ffea[el]
            s_valid[o:o + n] = True
        t_ids = np.arange(ec) // TILE
        rel = s_dstloc.astype(np.float32) - (t_ids // tpb) * BLK
        rel[~s_valid] = -1.0  # padding matches no indicator column
        dstrel_f = np.ascontiguousarray(rel.reshape(ec // TILE, TILE).T)
        shards.append(dict(
            src_pk=_pack_idxs(s_src),
            feaT=np.ascontiguousarray(s_fea.T).astype(NP_BF16),
            dstrel=dstrel_f.astype(np.float32),
            # broadcast layout: every partition row = per-edge dst-rel value
            dstrelB=np.tile(rel.astype(NP_BF16)[None, :], (128, 1)),
        ))
    return shards, tpb


# --------------------------------------------------------------- device build
def _build(tpb, nzb, dims=DIMS, compile=True):
    key = (tpb, nzb, tuple(sorted(dims.items())))
    if key in _BUILD_CACHE:
        return _BUILD_CACHE[key]
    N, NPC, BLK, NBLK = dims["N"], dims["NPC"], dims["BLK"], dims["NBLK"]
    nz_bf, nz_bl, nz_br, nz_bo = nzb
    epb = tpb * TILE
    ec = NBLK * epb

    nc = bacc.Bacc("TRN2", target_bir_lowering=False, debug=False, num_devices=C)
    d_xT = nc.dram_tensor("xT", [F_IN, NPC], bf16, kind="ExternalInput")
    d_feaT = nc.dram_tensor("feaT", [2, ec], bf16, kind="ExternalInput")
    d_srcpk = nc.dram_tensor("src_pk", [128, ec // 16], i16, kind="ExternalInput")
    d_dstrel = nc.dram_tensor("dstrel", [128, ec // TILE], f32, kind="ExternalInput")
    d_dstrelB = nc.dram_tensor("dstrelB", [128, ec], bf16, kind="ExternalInput")
    d_eye = nc.dram_tensor("eye", [128, 128], bf16, kind="ExternalInput")
    d_iorow = nc.dram_tensor("iorow", [128, BLK], bf16, kind="ExternalInput")
    d_iocol = nc.dram_tensor("iocol", [BLK, 1], f32, kind="ExternalInput")
    d_Wf = nc.dram_tensor("Wf", [F_IN, D], bf16, kind="ExternalInput")
    d_Wl = nc.dram_tensor("Wl", [D, HD], bf16, kind="ExternalInput")
    d_Wr = nc.dram_tensor("Wr", [D, HD], bf16, kind="ExternalInput")
    d_We = nc.dram_tensor("We", [2, HD], bf16, kind="ExternalInput")
    d_attb = nc.dram_tensor("att_b", [128, HD], bf16, kind="ExternalInput")
    d_bf = nc.dram_tensor("bf_col", [128, 1], f32, kind="ExternalInput")
    d_blb = nc.dram_tensor("bl_b", [128, HD], f32, kind="ExternalInput")
    d_brb = nc.dram_tensor("br_b", [128, HD], f32, kind="ExternalInput")
    d_bob = nc.dram_tensor("bo_b", [128, D], f32, kind="ExternalInput")
    d_out = nc.dram_tensor("hout", [NPC, D], f32, kind="ExternalOutput")

    with tile.TileContext(nc) as tc, ExitStack() as ex:
        cst = ex.enter_context(tc.tile_pool(name="cst", bufs=1))
        dram = ex.enter_context(tc.tile_pool(name="dram", bufs=1, space="DRAM"))
        ps512 = ex.enter_context(tc.tile_pool(name="ps512", bufs=2, space="PSUM"))
        psO = ex.enter_context(tc.tile_pool(name="psO", bufs=2, space="PSUM"))
        psD = ex.enter_context(tc.tile_pool(name="psD", bufs=1, space="PSUM"))
        psT = ex.enter_context(tc.tile_pool(name="psT", bufs=1, space="PSUM"))
        gb1 = ex.enter_context(tc.tile_pool(name="gb1", bufs=2))
        scr = ex.enter_context(tc.tile_pool(name="scr", bufs=3))
        blkp = ex.enter_context(tc.tile_pool(name="blkp", bufs=2))
        evp = ex.enter_context(tc.tile_pool(name="evp", bufs=3))
        feap = ex.enter_context(tc.tile_pool(name="feap", bufs=2))

        nc.gpsimd.load_library(mlp)

        def ld(dt, shape, dtype=bf16):
            t = cst.tile(shape, dtype, name=f"sb_{dt.name}")
            nc.sync.dma_start(t[:], dt[:])
            return t

        eye = ld(d_eye, [128, 128])
        iorow = ld(d_iorow, [128, BLK])
        iocol = ld(d_iocol, [BLK, 1], f32)
        Wf = ld(d_Wf, [F_IN, D])
        Wl = ld(d_Wl, [D, HD])
        Wr = ld(d_Wr, [D, HD])
        We = ld(d_We, [2, HD])
        attb = ld(d_attb, [128, HD])
        xT = ld(d_xT, [F_IN, NPC])
        srcpk = ld(d_srcpk, [128, ec // 16], i16)
        dstrel = ld(d_dstrel, [128, ec // TILE], f32)
        dstrelB = ld(d_dstrelB, [128, ec])
        bf = ld(d_bf, [128, 1], f32) if nz_bf else None
        blb = ld(d_blb, [128, HD], f32) if nz_bl else None
        brb = ld(d_brb, [128, HD], f32) if nz_br else None
        bob = ld(d_bob, [128, D], f32) if nz_bo else None

        alpha_c = cst.tile([128, 1], f32, name="alpha_c")
        nc.vector.memset(alpha_c[:], NEG)
        hT = cst.tile([128, NPC], bf16, name="hT")
        xr_all = cst.tile([BLK, NBLK, HD], bf16, name="xr_all")
        agins = [dram.tile([NPC, HD], bf16, name=f"agin{i}")
                 for i in range(N_LAYERS)]
        agouts = [dram.tile([N, HD], bf16, addr_space="Shared", name=f"agout{i}")
                  for i in range(N_LAYERS)]

        # ---- layer-0 features, feature-major: h0T = Wf.T @ xT (+ bf)
        CH = min(NPC, 500)
        assert NPC % CH == 0
        for j in range(NPC // CH):
            ps = ps512.tile([128, CH], f32, tag="ps512")
            nc.tensor.matmul(ps[:], Wf[:], xT[:, j * CH:(j + 1) * CH],
                             start=True, stop=True)
            dst = hT[:, j * CH:(j + 1) * CH]
            if nz_bf:
                nc.vector.tensor_scalar_add(dst, ps[:], bf[:])
            else:
                nc.scalar.copy(dst, ps[:])

        for L in range(N_LAYERS):
            agin, agout = agins[L], agouts[L]
            # ---- node matmuls (own shard) -> xl to agin, xr to SBUF xr_all
            for m in range(NBLK):
                lh = hT[:, m * BLK:(m + 1) * BLK]
                psl = ps512.tile([BLK, HD], f32, tag="ps512")
                nc.tensor.matmul(psl[:], lh, Wl[:], start=True, stop=True)
                xle = evp.tile([BLK, HD], bf16, tag="ev")
                if nz_bl:
                    nc.vector.tensor_add(xle[:], psl[:], blb[:BLK, :])
                else:
                    nc.vector.tensor_copy(xle[:], psl[:])
                nc.sync.dma_start(agin[m * BLK:(m + 1) * BLK, :], xle[:])
                psr = ps512.tile([BLK, HD], f32, tag="ps512")
                nc.tensor.matmul(psr[:], lh, Wr[:], start=True, stop=True)
                if nz_br:
                    nc.vector.tensor_add(xr_all[:, m, :], psr[:], brb[:BLK, :])
                else:
                    nc.vector.tensor_copy(xr_all[:, m, :], psr[:])

            nc.gpsimd.collective_compute(
                "AllGather", ALU.bypass,
                replica_groups=[list(range(C))],
                ins=[agin.opt()], outs=[agout.opt()],
            )

            # ---- edge phase, per dst block
            for b in range(NBLK):
                e0 = b * epb
                GC = 4  # tiles per gather call (512 idxs: SWDGE ring limit)
                xlg = gb1.tile([128, tpb, HD], bf16, tag="xlg")
                for g0 in range(0, tpb, GC):
                    g1 = min(g0 + GC, tpb)
                    ne = (g1 - g0) * TILE
                    c0 = (e0 + g0 * TILE) // 16
                    nc.gpsimd.dma_gather(xlg[:, g0:g1, :], agout[:],
                                         srcpk[:, c0:c0 + ne // 16],
                                         ne, ne, HD)
                feaT = feap.tile([2, epb], bf16, tag="feaT")
                nc.sync.dma_start(feaT[:], d_feaT[:, e0:e0 + epb])
                # node-major indicator for the whole block: B[n,e]=1 iff dst(e)=n
                Bn = blkp.tile([BLK, tpb, TILE], bf16, tag="Bn")
                nc.vector.tensor_scalar(
                    Bn[:].rearrange("p a b -> p (a b)"),
                    dstrelB[:BLK, e0:e0 + epb], iocol[:], None, ALU.is_equal)
                lgb = blkp.tile([128, tpb, H], f32, tag="lgb")
                indb = blkp.tile([128, tpb, BLK], bf16, tag="indb")
                assert tpb % 2 == 0
                for t0 in range(0, tpb, 2):
                    zp = ps512.tile([128, 2, HD], f32, tag="ps512")
                    for dt_ in range(2):
                        t = t0 + dt_
                        nc.vector.tensor_scalar(
                            indb[:, t, :], iorow[:],
                            dstrel[:, b * tpb + t:b * tpb + t + 1], None,
                            ALU.is_equal)
                        nc.tensor.matmul(zp[:, dt_, :],
                                         feaT[:, t * TILE:(t + 1) * TILE],
                                         We[:], start=True, stop=False)
                        nc.tensor.matmul(zp[:, dt_, :], Bn[:, t, :],
                                         xr_all[:, b, :],
                                         start=False, stop=False)
                    for dt_ in range(2):
                        nc.tensor.matmul(zp[:, dt_, :], eye[:],
                                         xlg[:, t0 + dt_, :],
                                         start=False, stop=True)
                    lz = scr.tile([128, 2, HD], bf16, tag="lz")
                    nc.scalar.activation(
                        lz[:].rearrange("p a b -> p (a b)"),
                        zp[:].rearrange("p a b -> p (a b)"), AF.Prelu,
                        alpha=alpha_c[:])
                    y = scr.tile([128, 2, HD], bf16, tag="y")
                    for dt_ in range(2):
                        nc.vector.tensor_mul(y[:, dt_, :], lz[:, dt_, :],
                                             attb[:])
                    nc.vector.tensor_reduce(
                        lgb[:, t0:t0 + 2, :],
                        y[:].rearrange("p a (h d) -> p (a h) d", h=H),
                        axis=mybir.AxisListType.X, op=ALU.add)
                webf = blkp.tile([128, tpb, H], f32, tag="webf")
                nc.scalar.activation(webf[:], lgb[:], AF.Exp)
                web = blkp.tile([128, tpb, H], bf16, tag="web")
                nc.scalar.activation(web[:], lgb[:], AF.Exp)
                outp = psO.tile([BLK, HD], f32, tag="psO")
                denp = psD.tile([BLK, H], f32, tag="psD")
                for t in range(tpb):
                    msg = scr.tile([128, HD], bf16, tag="msg")
                    for hh in range(H):
                        if hh < 2:
                            nc.scalar.activation(
                                msg[:, hh * D:(hh + 1) * D],
                                xlg[:, t, hh * D:(hh + 1) * D],
                                AF.Copy, scale=webf[:, t, hh:hh + 1])
                        else:
                            nc.vector.tensor_scalar_mul(
                                msg[:, hh * D:(hh + 1) * D],
                                xlg[:, t, hh * D:(hh + 1) * D],
                                webf[:, t, hh:hh + 1])
                    nc.tensor.matmul(outp[:], indb[:, t, :], msg[:],
                                     start=(t == 0), stop=(t == tpb - 1))
                    nc.tensor.matmul(denp[:], indb[:, t, :], web[:, t, :],
                                     start=(t == 0), stop=(t == tpb - 1))
                invd = blkp.tile([BLK, H], f32, tag="invd")
                nc.vector.reciprocal(invd[:], denp[:])
                # fold the head-mean 1/H into the normalizer
                nc.vector.tensor_scalar_mul(invd[:], invd[:], 1.0 / H)
                o = blkp.tile([BLK, HD], bf16, tag="o")
                for hh in range(H):
                    nc.scalar.activation(
                        o[:, hh * D:(hh + 1) * D],
                        outp[:, hh * D:(hh + 1) * D],
                        AF.Copy, scale=invd[:, hh:hh + 1])
                s01 = blkp.tile([BLK, D], bf16, tag="s01")
                nc.vector.tensor_add(s01[:], o[:, 0:D], o[:, D:2 * D])
                s23 = blkp.tile([BLK, D], bf16, tag="s23")
                nc.vector.tensor_add(s23[:], o[:, 2 * D:3 * D], o[:, 3 * D:4 * D])
                sm = blkp.tile([BLK, D], bf16, tag="sm")
                if nz_bo:
                    nc.vector.tensor_add(sm[:], s01[:], s23[:])
                    nc.vector.tensor_add(sm[:], sm[:], bob[:BLK, :])
                else:
                    nc.vector.tensor_add(sm[:], s01[:], s23[:])
                if L == N_LAYERS - 1:
                    hb = blkp.tile([BLK, D], f32, tag="hbf")
                    nc.vector.scalar_tensor_tensor(
                        hb[:], sm[:], 0.01, sm[:], ALU.mult, ALU.max)
                    nc.sync.dma_start(d_out[b * BLK:(b + 1) * BLK, :], hb[:])
                else:
                    hb = blkp.tile([BLK, D], bf16, tag="hb")
                    nc.vector.scalar_tensor_tensor(
                        hb[:], sm[:], 0.01, sm[:], ALU.mult, ALU.max)
                    tp = psT.tile([128, BLK], bf16, tag="psT")
                    nc.tensor.transpose(tp[:], hb[:], eye[:BLK, :BLK])
                    nc.scalar.copy(hT[:, b * BLK:(b + 1) * BLK], tp[:])

    if compile:
        nc.compile()
    _BUILD_CACHE[key] = nc
    return nc


# ------------------------------------------------------------------ in_maps
def make_in_maps(inputs, dims=DIMS):
    N, NPC, BLK = dims["N"], dims["NPC"], dims["BLK"]
    x = np.asarray(inputs["x"], np.float32)
    Wf = np.ascontiguousarray(np.asarray(inputs["Wf"], np.float32))
    bf = np.asarray(inputs["bf"], np.float32)
    Wl = np.ascontiguousarray(np.asarray(inputs["Wl"], np.float32))
    bl = np.asarray(inputs["bl"], np.float32)
    Wr = np.ascontiguousarray(np.asarray(inputs["Wr"], np.float32))
    br = np.asarray(inputs["br"], np.float32)
    We = np.ascontiguousarray(np.asarray(inputs["We"], np.float32))
    att = np.asarray(inputs["att"], np.float32)
    bias_out = np.asarray(inputs["bias_out"], np.float32)

    shards, tpb = _build_shards(inputs["edge_index"], inputs["edge_attr"], dims)
    nzb = (bool(bf.any()), bool(bl.any()), bool(br.any()), bool(bias_out.any()))

    common = dict(
        eye=np.eye(128, dtype=NP_BF16),
        iorow=np.tile(np.arange(BLK, dtype=NP_BF16), (128, 1)),
        iocol=np.arange(BLK, dtype=np.float32).reshape(BLK, 1),
        Wf=Wf.astype(NP_BF16), Wl=Wl.astype(NP_BF16),
        Wr=Wr.astype(NP_BF16), We=We.astype(NP_BF16),
        att_b=np.tile(att.reshape(1, HD), (128, 1)).astype(NP_BF16),
        bf_col=np.ascontiguousarray(bf.reshape(D, 1)),
        bl_b=np.tile(bl.reshape(1, HD), (128, 1)).astype(np.float32),
        br_b=np.tile(br.reshape(1, HD), (128, 1)).astype(np.float32),
        bo_b=np.tile(bias_out.reshape(1, D), (128, 1)).astype(np.float32),
    )
    in_maps = []
    for c in range(C):
        sh = shards[c]
        m = dict(common)
        m["xT"] = np.ascontiguousarray(x[c * NPC:(c + 1) * NPC].T).astype(NP_BF16)
        m["feaT"] = sh["feaT"]
        m["src_pk"] = sh["src_pk"]
        m["dstrel"] = sh["dstrel"]
        m["dstrelB"] = sh["dstrelB"]
        in_maps.append(m)
    return in_maps, tpb, nzb


# -------------------------------------------------------------- bench hooks
def build_for_inputs(inputs):
    in_maps, tpb, nzb = make_in_maps(inputs, DIMS)
    nc = _build(tpb, nzb, DIMS)
    return nc, in_maps


def assemble_output(outs, out_names):
    NPC = DIMS["NPC"]
    got = np.asarray(outs[out_names.index("hout")]).reshape(C, NPC, -1)
    return got.reshape(C * NPC, -1).astype(np.float32)


# -------------------------------------------------------------------- kernel
def kernel(**inputs):
    in_maps, tpb, nzb = make_in_maps(inputs, DIMS)
    nc = _build(tpb, nzb, DIMS)
    res = run_bass_kernel_spmd(nc, in_maps, list(range(C)))
    NPC = DIMS["NPC"]
    return np.concatenate([res.results[c]["hout"] for c in range(C)], axis=0)


if __name__ == "__main__":
    nc = _build(10, (False, False, False, False), DIMS, compile=False)
    n_inst = sum(len(f.blocks[0].instructions) for f in nc.m.functions)
    print("trace-only build OK")


# revision 13
# speedup vs baseline: 3.4548x; 1.0044x over previous
"""GATv2 (3-layer, 4-head) on 8 Trainium2 NeuronCores — Bass/Tile SPMD kernel.

Sharding: destination-node partition (graph parallel). Core c owns dst nodes
[c*NPC, (c+1)*NPC) in NBLK blocks of BLK. Edges (incl. mean-filled
self-loops) are bucketed by dst block; all cores run one shared SPMD
program over padded, per-core index data.

Compute dtype is bf16 (PSUM accumulation fp32). Per layer:
  1. sharded node matmuls xl = h@Wl (to DRAM for AllGather) and
     xr = h@Wr (kept in SBUF — dst rows are block-local)
  2. AllGather of the bf16 xl table (only collective)
  3. per dst-block: dma_gather xl[src] rows only; xr[dst] is assembled
     with a node-major one-hot indicator matmul from the SBUF xr block;
     z = ee + xl_g + xr_g accumulated in PSUM,
     leaky_relu via one scalar_tensor_tensor on DVE, att-dot + per-head
     reduce on DVE, exp on ScalarE (softmax max-shift dropped: alpha is
     shift-invariant, logits are O(1)),
     msg = w * xl_g via ScalarE activation-with-scale,
     unnormalized scatter out += A^T @ msg and denom += A^T @ w
     via edge-major one-hot indicator matmuls,
     then normalize via ScalarE activation-with-scale (folding 1/H),
     head-mean, bias, outer leaky_relu.
"""
import sys

sys.path.insert(0, "/opt/trn_rl_repo")
from contextlib import ExitStack

import numpy as np
import concourse.bacc as bacc
import concourse.mybir as mybir
import concourse.tile as tile
from concourse.bass_utils import run_bass_kernel_spmd
from concourse.library_config import mlp

f32 = mybir.dt.float32
bf16 = mybir.dt.bfloat16
i16 = mybir.dt.int16
ALU = mybir.AluOpType
AF = mybir.ActivationFunctionType
NP_BF16 = mybir.dt.np(bf16)

H = 4
D = 128
HD = H * D
F_IN = 128
NEG = 0.2
N_LAYERS = 3
C = 8
TILE = 128

# full-problem dims (overridable for small-scale sim tests)
DIMS = dict(N=20000, NPC=2500, BLK=125, NBLK=20)

_BUILD_CACHE = {}


# ----------------------------------------------------------------- host prep
def _pack_idxs(il):
    n = len(il)
    a = np.zeros((128, n // 16), np.int16)
    base = il.reshape(n // 16, 16).T
    for g in range(8):
        a[g * 16:(g + 1) * 16] = base
    return a


def _build_shards(edge_index, edge_attr, dims=DIMS):
    N, NPC, BLK, NBLK = dims["N"], dims["NPC"], dims["BLK"], dims["NBLK"]
    src = np.asarray(edge_index[0], np.int64)
    dst = np.asarray(edge_index[1], np.int64)
    ea = np.asarray(edge_attr, np.float32)

    ea_sum = np.zeros((N, 2), np.float32)
    np.add.at(ea_sum, dst, ea)
    cnt = np.bincount(dst, minlength=N).astype(np.float32)
    loop_attr = ea_sum / np.maximum(cnt, 1.0)[:, None]

    fsrc = np.concatenate([src, np.arange(N, dtype=np.int64)])
    fdst = np.concatenate([dst, np.arange(N, dtype=np.int64)])
    ffea = np.concatenate([ea, loop_attr], axis=0)

    key = fdst // NPC * NBLK + (fdst % NPC) // BLK
    order = np.argsort(key, kind="stable")
    kb = key[order]
    bounds = np.searchsorted(kb, np.arange(C * NBLK + 1))
    max_edges = int(np.max(np.diff(bounds)))
    tpb = (max_edges + TILE - 1) // TILE
    epb = tpb * TILE
    ec = NBLK * epb

    shards = []
    for c in range(C):
        s_src = np.zeros(ec, np.int16)
        s_dstloc = np.zeros(ec, np.int16)
        s_fea = np.zeros((ec, 2), np.float32)
        s_valid = np.zeros(ec, bool)
        for b in range(NBLK):
            k = c * NBLK + b
            el = order[bounds[k]:bounds[k + 1]]
            o = b * epb
            n = len(el)
            s_src[o:o + n] = fsrc[el].astype(np.int16)
            s_dstloc[o:o + n] = (fdst[el] - c * NPC).astype(np.int16)
            s_fea[o:o + n] = ffea[el]
            s_valid[o:o + n] = True
        t_ids = np.arange(ec) // TILE
        rel = s_dstloc.astype(np.float32) - (t_ids // tpb) * BLK
        rel[~s_valid] = -1.0  # padding matches no indicator column
        dstrel_f = np.ascontiguousarray(rel.reshape(ec // TILE, TILE).T)
        shards.append(dict(
            src_pk=_pack_idxs(s_src),
            feaT=np.ascontiguousarray(s_fea.T).astype(NP_BF16),
            dstrel=dstrel_f.astype(np.float32),
            # broadcast layout: every partition row = per-edge dst-rel value
            dstrelB=np.tile(rel.astype(NP_BF16)[None, :], (128, 1)),
        ))
    return shards, tpb


# --------------------------------------------------------------- device build
def _build(tpb, nzb, dims=DIMS, compile=True):
    key = (tpb, nzb, tuple(sorted(dims.items())))
    if key in _BUILD_CACHE:
        return _BUILD_CACHE[key]
    N, NPC, BLK, NBLK = dims["N"], dims["NPC"], dims["BLK"], dims["NBLK"]
    nz_bf, nz_bl, nz_br, nz_bo = nzb
    epb = tpb * TILE
    ec = NBLK * epb

    nc = bacc.Bacc("TRN2", target_bir_lowering=False, debug=False, num_devices=C)
    d_xT = nc.dram_tensor("xT", [F_IN, NPC], bf16, kind="ExternalInput")
    d_feaT = nc.dram_tensor("feaT", [2, ec], bf16, kind="ExternalInput")
    d_srcpk = nc.dram_tensor("src_pk", [128, ec // 16], i16, kind="ExternalInput")
    d_dstrel = nc.dram_tensor("dstrel", [128, ec // TILE], f32, kind="ExternalInput")
    d_dstrelB = nc.dram_tensor("dstrelB", [128, ec], bf16, kind="ExternalInput")
    d_eye = nc.dram_tensor("eye", [128, 128], bf16, kind="ExternalInput")
    d_iorow = nc.dram_tensor("iorow", [128, BLK], bf16, kind="ExternalInput")
    d_iocol = nc.dram_tensor("iocol", [BLK, 1], f32, kind="ExternalInput")
    d_Wf = nc.dram_tensor("Wf", [F_IN, D], bf16, kind="ExternalInput")
    d_Wl = nc.dram_tensor("Wl", [D, HD], bf16, kind="ExternalInput")
    d_Wr = nc.dram_tensor("Wr", [D, HD], bf16, kind="ExternalInput")
    d_We = nc.dram_tensor("We", [2, HD], bf16, kind="ExternalInput")
    d_attb = nc.dram_tensor("att_b", [128, HD], bf16, kind="ExternalInput")
    d_bf = nc.dram_tensor("bf_col", [128, 1], f32, kind="ExternalInput")
    d_blb = nc.dram_tensor("bl_b", [128, HD], f32, kind="ExternalInput")
    d_brb = nc.dram_tensor("br_b", [128, HD], f32, kind="ExternalInput")
    d_bob = nc.dram_tensor("bo_b", [128, D], f32, kind="ExternalInput")
    d_out = nc.dram_tensor("hout", [NPC, D], f32, kind="ExternalOutput")

    with tile.TileContext(nc) as tc, ExitStack() as ex:
        cst = ex.enter_context(tc.tile_pool(name="cst", bufs=1))
        dram = ex.enter_context(tc.tile_pool(name="dram", bufs=1, space="DRAM"))
        ps512 = ex.enter_context(tc.tile_pool(name="ps512", bufs=2, space="PSUM"))
        psO = ex.enter_context(tc.tile_pool(name="psO", bufs=2, space="PSUM"))
        psD = ex.enter_context(tc.tile_pool(name="psD", bufs=1, space="PSUM"))
        psT = ex.enter_context(tc.tile_pool(name="psT", bufs=1, space="PSUM"))
        gb1 = ex.enter_context(tc.tile_pool(name="gb1", bufs=2))
        scr = ex.enter_context(tc.tile_pool(name="scr", bufs=3))
        blkp = ex.enter_context(tc.tile_pool(name="blkp", bufs=2))
        evp = ex.enter_context(tc.tile_pool(name="evp", bufs=3))
        feap = ex.enter_context(tc.tile_pool(name="feap", bufs=2))

        nc.gpsimd.load_library(mlp)

        def ld(dt, shape, dtype=bf16):
            t = cst.tile(shape, dtype, name=f"sb_{dt.name}")
            nc.sync.dma_start(t[:], dt[:])
            return t

        eye = ld(d_eye, [128, 128])
        iorow = ld(d_iorow, [128, BLK])
        iocol = ld(d_iocol, [BLK, 1], f32)
        Wf = ld(d_Wf, [F_IN, D])
        Wl = ld(d_Wl, [D, HD])
        Wr = ld(d_Wr, [D, HD])
        We = ld(d_We, [2, HD])
        attb = ld(d_attb, [128, HD])
        xT = ld(d_xT, [F_IN, NPC])
        srcpk = ld(d_srcpk, [128, ec // 16], i16)
        dstrel = ld(d_dstrel, [128, ec // TILE], f32)
        dstrelB = ld(d_dstrelB, [128, ec])
        bf = ld(d_bf, [128, 1], f32) if nz_bf else None
        blb = ld(d_blb, [128, HD], f32) if nz_bl else None
        brb = ld(d_brb, [128, HD], f32) if nz_br else None
        bob = ld(d_bob, [128, D], f32) if nz_bo else None

        alpha_c = cst.tile([128, 1], f32, name="alpha_c")
        nc.vector.memset(alpha_c[:], NEG)
        hT = cst.tile([128, NPC], bf16, name="hT")
        xr_all = cst.tile([BLK, NBLK, HD], bf16, name="xr_all")
        agins = [dram.tile([NPC, HD], bf16, name=f"agin{i}")
                 for i in range(N_LAYERS)]
        agouts = [dram.tile([N, HD], bf16, addr_space="Shared", name=f"agout{i}")
                  for i in range(N_LAYERS)]

        # ---- layer-0 features, feature-major: h0T = Wf.T @ xT (+ bf)
        CH = min(NPC, 500)
        assert NPC % CH == 0
        for j in range(NPC // CH):
            ps = ps512.tile([128, CH], f32, tag="ps512")
            nc.tensor.matmul(ps[:], Wf[:], xT[:, j * CH:(j + 1) * CH],
                             start=True, stop=True)
            dst = hT[:, j * CH:(j + 1) * CH]
            if nz_bf:
                nc.vector.tensor_scalar_add(dst, ps[:], bf[:])
            else:
                nc.scalar.copy(dst, ps[:])

        def node_mms(j, m, pool):
            # xl/xr of layer j for node block m (uses current hT state)
            lh = hT[:, m * BLK:(m + 1) * BLK]
            psl = pool.tile([BLK, HD], f32, tag=pool.name)
            nc.tensor.matmul(psl[:], lh, Wl[:], start=True, stop=True)
            xle = evp.tile([BLK, HD], bf16, tag="ev")
            if nz_bl:
                nc.vector.tensor_add(xle[:], psl[:], blb[:BLK, :])
            else:
                nc.scalar.copy(xle[:], psl[:])
            nc.sync.dma_start(agins[j][m * BLK:(m + 1) * BLK, :], xle[:])
            psr = pool.tile([BLK, HD], f32, tag=pool.name)
            nc.tensor.matmul(psr[:], lh, Wr[:], start=True, stop=True)
            if nz_br:
                nc.vector.tensor_add(xr_all[:, m, :], psr[:], brb[:BLK, :])
            else:
                nc.scalar.copy(xr_all[:, m, :], psr[:])

        def full_ag(j):
            nc.gpsimd.collective_compute(
                "AllGather", ALU.bypass,
                replica_groups=[list(range(C))],
                ins=[agins[j].opt()], outs=[agouts[j].opt()],
            )

        for L in range(N_LAYERS):
            agout = agouts[L]
            if L == 0:
                # layer 0: standalone node phase + both collective halves
                for m in range(NBLK):
                    node_mms(0, m, ps512)
                full_ag(0)

            # ---- edge phase, per dst block
            for b in range(NBLK):
                e0 = b * epb
                GC = 4  # tiles per gather call (512 idxs: SWDGE ring limit)
                xlg = gb1.tile([128, tpb, HD], bf16, tag="xlg")
                for g0 in range(0, tpb, GC):
                    g1 = min(g0 + GC, tpb)
                    ne = (g1 - g0) * TILE
                    c0 = (e0 + g0 * TILE) // 16
                    nc.gpsimd.dma_gather(xlg[:, g0:g1, :], agout[:],
                                         srcpk[:, c0:c0 + ne // 16],
                                         ne, ne, HD)
                feaT = feap.tile([2, epb], bf16, tag="feaT")
                nc.sync.dma_start(feaT[:], d_feaT[:, e0:e0 + epb])
                # node-major indicator for the whole block: B[n,e]=1 iff dst(e)=n
                Bn = blkp.tile([BLK, tpb, TILE], bf16, tag="Bn")
                nc.vector.tensor_scalar(
                    Bn[:].rearrange("p a b -> p (a b)"),
                    dstrelB[:BLK, e0:e0 + epb], iocol[:], None, ALU.is_equal)
                lgb = blkp.tile([128, tpb, H], f32, tag="lgb")
                indb = blkp.tile([128, tpb, BLK], bf16, tag="indb")
                assert tpb % 2 == 0
                for t0 in range(0, tpb, 2):
                    zp = ps512.tile([128, 2, HD], f32, tag="ps512")
                    for dt_ in range(2):
                        t = t0 + dt_
                        nc.vector.tensor_scalar(
                            indb[:, t, :], iorow[:],
                            dstrel[:, b * tpb + t:b * tpb + t + 1], None,
                            ALU.is_equal)
                        nc.tensor.matmul(zp[:, dt_, :],
                                         feaT[:, t * TILE:(t + 1) * TILE],
                                         We[:], start=True, stop=False)
                        nc.tensor.matmul(zp[:, dt_, :], Bn[:, t, :],
                                         xr_all[:, b, :],
                                         start=False, stop=False)
                    for dt_ in range(2):
                        nc.tensor.matmul(zp[:, dt_, :], eye[:],
                                         xlg[:, t0 + dt_, :],
                                         start=False, stop=True)
                    lz = scr.tile([128, 2, HD], bf16, tag="lz")
                    nc.scalar.activation(
                        lz[:].rearrange("p a b -> p (a b)"),
                        zp[:].rearrange("p a b -> p (a b)"), AF.Prelu,
                        alpha=alpha_c[:])
                    y = scr.tile([128, 2, HD], bf16, tag="y")
                    for dt_ in range(2):
                        nc.vector.tensor_mul(y[:, dt_, :], lz[:, dt_, :],
                                             attb[:])
                    nc.vector.tensor_reduce(
                        lgb[:, t0:t0 + 2, :],
                        y[:].rearrange("p a (h d) -> p (a h) d", h=H),
                        axis=mybir.AxisListType.X, op=ALU.add)
                webf = blkp.tile([128, tpb, H], f32, tag="webf")
                nc.scalar.activation(webf[:], lgb[:], AF.Exp)
                web = blkp.tile([128, tpb, H], bf16, tag="web")
                nc.scalar.activation(web[:], lgb[:], AF.Exp)
                outp = psO.tile([BLK, HD], f32, tag="psO")
                denp = psD.tile([BLK, H], f32, tag="psD")
                for t in range(tpb):
                    msg = scr.tile([128, HD], bf16, tag="msg")
                    for hh in range(H):
                        if hh < 2:
                            nc.scalar.activation(
                                msg[:, hh * D:(hh + 1) * D],
                                xlg[:, t, hh * D:(hh + 1) * D],
                                AF.Copy, scale=webf[:, t, hh:hh + 1])
                        else:
                            nc.vector.tensor_scalar_mul(
                                msg[:, hh * D:(hh + 1) * D],
                                xlg[:, t, hh * D:(hh + 1) * D],
                                webf[:, t, hh:hh + 1])
                    nc.tensor.matmul(outp[:], indb[:, t, :], msg[:],
                                     start=(t == 0), stop=(t == tpb - 1))
                    nc.tensor.matmul(denp[:], indb[:, t, :], web[:, t, :],
                                     start=(t == 0), stop=(t == tpb - 1))
                invd = blkp.tile([BLK, H], f32, tag="invd")
                nc.vector.reciprocal(invd[:], denp[:])
                # fold the head-mean 1/H into the normalizer
                nc.vector.tensor_scalar_mul(invd[:], invd[:], 1.0 / H)
                o = blkp.tile([BLK, HD], bf16, tag="o")
                for hh in range(H):
                    nc.scalar.activation(
                        o[:, hh * D:(hh + 1) * D],
                        outp[:, hh * D:(hh + 1) * D],
                        AF.Copy, scale=invd[:, hh:hh + 1])
                s01 = blkp.tile([BLK, D], bf16, tag="s01")
                nc.vector.tensor_add(s01[:], o[:, 0:D], o[:, D:2 * D])
                s23 = blkp.tile([BLK, D], bf16, tag="s23")
                nc.vector.tensor_add(s23[:], o[:, 2 * D:3 * D], o[:, 3 * D:4 * D])
                sm = blkp.tile([BLK, D], bf16, tag="sm")
                if nz_bo:
                    nc.vector.tensor_add(sm[:], s01[:], s23[:])
                    nc.vector.tensor_add(sm[:], sm[:], bob[:BLK, :])
                else:
                    nc.vector.tensor_add(sm[:], s01[:], s23[:])
                if L == N_LAYERS - 1:
                    hb = blkp.tile([BLK, D], f32, tag="hbf")
                    nc.vector.scalar_tensor_tensor(
                        hb[:], sm[:], 0.01, sm[:], ALU.mult, ALU.max)
                    nc.sync.dma_start(d_out[b * BLK:(b + 1) * BLK, :], hb[:])
                else:
                    hb = blkp.tile([BLK, D], bf16, tag="hb")
                    nc.vector.scalar_tensor_tensor(
                        hb[:], sm[:], 0.01, sm[:], ALU.mult, ALU.max)
                    tp = psT.tile([128, BLK], bf16, tag="psT")
                    nc.tensor.transpose(tp[:], hb[:], eye[:BLK, :BLK])
                    nc.scalar.copy(hT[:, b * BLK:(b + 1) * BLK], tp[:])
                    # next layer's node matmuls for this block, so the
                    # AllGather halves launch mid-edge-phase and overlap
                    node_mms(L + 1, b, psT)
                    if b == NBLK - 1:
                        full_ag(L + 1)

    if compile:
        nc.compile()
    _BUILD_CACHE[key] = nc
    return nc


# ------------------------------------------------------------------ in_maps
def make_in_maps(inputs, dims=DIMS):
    N, NPC, BLK = dims["N"], dims["NPC"], dims["BLK"]
    x = np.asarray(inputs["x"], np.float32)
    Wf = np.ascontiguousarray(np.asarray(inputs["Wf"], np.float32))
    bf = np.asarray(inputs["bf"], np.float32)
    Wl = np.ascontiguousarray(np.asarray(inputs["Wl"], np.float32))
    bl = np.asarray(inputs["bl"], np.float32)
    Wr = np.ascontiguousarray(np.asarray(inputs["Wr"], np.float32))
    br = np.asarray(inputs["br"], np.float32)
    We = np.ascontiguousarray(np.asarray(inputs["We"], np.float32))
    att = np.asarray(inputs["att"], np.float32)
    bias_out = np.asarray(inputs["bias_out"], np.float32)

    shards, tpb = _build_shards(inputs["edge_index"], inputs["edge_attr"], dims)
    nzb = (bool(bf.any()), bool(bl.any()), bool(br.any()), bool(bias_out.any()))

    common = dict(
        eye=np.eye(128, dtype=NP_BF16),
        iorow=np.tile(np.arange(BLK, dtype=NP_BF16), (128, 1)),
        iocol=np.arange(BLK, dtype=np.float32).reshape(BLK, 1),
        Wf=Wf.astype(NP_BF16), Wl=Wl.astype(NP_BF16),
        Wr=Wr.astype(NP_BF16), We=We.astype(NP_BF16),
        att_b=np.tile(att.reshape(1, HD), (128, 1)).astype(NP_BF16),
        bf_col=np.ascontiguousarray(bf.reshape(D, 1)),
        bl_b=np.tile(bl.reshape(1, HD), (128, 1)).astype(np.float32),
        br_b=np.tile(br.reshape(1, HD), (128, 1)).astype(np.float32),
        bo_b=np.tile(bias_out.reshape(1, D), (128, 1)).astype(np.float32),
    )
    in_maps = []
    for c in range(C):
        sh = shards[c]
        m = dict(common)
        m["xT"] = np.ascontiguousarray(x[c * NPC:(c + 1) * NPC].T).astype(NP_BF16)
        m["feaT"] = sh["feaT"]
        m["src_pk"] = sh["src_pk"]
        m["dstrel"] = sh["dstrel"]
        m["dstrelB"] = sh["dstrelB"]
        in_maps.append(m)
    return in_maps, tpb, nzb


# -------------------------------------------------------------- bench hooks
def build_for_inputs(inputs):
    in_maps, tpb, nzb = make_in_maps(inputs, DIMS)
    nc = _build(tpb, nzb, DIMS)
    return nc, in_maps


def assemble_output(outs, out_names):
    NPC = DIMS["NPC"]
    got = np.asarray(outs[out_names.index("hout")]).reshape(C, NPC, -1)
    return got.reshape(C * NPC, -1).astype(np.float32)


# -------------------------------------------------------------------- kernel
def kernel(**inputs):
    in_maps, tpb, nzb = make_in_maps(inputs, DIMS)
    nc = _build(tpb, nzb, DIMS)
    res = run_bass_kernel_spmd(nc, in_maps, list(range(C)))
    NPC = DIMS["NPC"]
    return np.concatenate([res.results[c]["hout"] for c in range(C)], axis=0)


if __name__ == "__main__":
    nc = _build(10, (False, False, False, False), DIMS, compile=False)
    n_inst = sum(len(f.blocks[0].instructions) for f in nc.m.functions)
    print("trace-only build OK")


# revision 15
# speedup vs baseline: 4.1150x; 1.1911x over previous
"""GATv2 (3-layer, 4-head) on 8 Trainium2 NeuronCores — Bass/Tile SPMD kernel.

Sharding: destination-node partition (graph parallel). Core c owns dst nodes
[c*NPC, (c+1)*NPC) in NBLK blocks of BLK. Edges (incl. mean-filled
self-loops) are bucketed by dst block; all cores run one shared SPMD
program over padded, per-core index data.

Compute dtype is bf16 (PSUM accumulation fp32). Per layer:
  1. sharded node matmuls xl = h@Wl (to DRAM for AllGather) and
     xr = h@Wr (kept in SBUF — dst rows are block-local)
  2. AllGather of the bf16 xl table (only collective)
  3. per dst-block: dma_gather xl[src] rows only; xr[dst] is assembled
     with a node-major one-hot indicator matmul from the SBUF xr block;
     z = ee + xl_g + xr_g accumulated in PSUM,
     leaky_relu via one scalar_tensor_tensor on DVE, att-dot + per-head
     reduce on DVE, exp on ScalarE (softmax max-shift dropped: alpha is
     shift-invariant, logits are O(1)),
     msg = w * xl_g via ScalarE activation-with-scale,
     unnormalized scatter out += A^T @ msg and denom += A^T @ w
     via edge-major one-hot indicator matmuls,
     then normalize via ScalarE activation-with-scale (folding 1/H),
     head-mean, bias, outer leaky_relu.
"""
import sys

sys.path.insert(0, "/opt/trn_rl_repo")
from contextlib import ExitStack

import numpy as np
import concourse.bacc as bacc
import concourse.mybir as mybir
import concourse.tile as tile
from concourse.bass_utils import run_bass_kernel_spmd
from concourse.library_config import mlp

f32 = mybir.dt.float32
bf16 = mybir.dt.bfloat16
i16 = mybir.dt.int16
ALU = mybir.AluOpType
AF = mybir.ActivationFunctionType
NP_BF16 = mybir.dt.np(bf16)

H = 4
D = 128
HD = H * D
F_IN = 128
NEG = 0.2
N_LAYERS = 3
C = 8
TILE = 128

# full-problem dims (overridable for small-scale sim tests)
DIMS = dict(N=20000, NPC=2500, BLK=125, NBLK=20)

_BUILD_CACHE = {}


# ----------------------------------------------------------------- host prep
def _pack_idxs(il):
    n = len(il)
    a = np.zeros((128, n // 16), np.int16)
    base = il.reshape(n // 16, 16).T
    for g in range(8):
        a[g * 16:(g + 1) * 16] = base
    return a


def _build_shards(edge_index, edge_attr, dims=DIMS):
    N, NPC, BLK, NBLK = dims["N"], dims["NPC"], dims["BLK"], dims["NBLK"]
    src = np.asarray(edge_index[0], np.int64)
    dst = np.asarray(edge_index[1], np.int64)
    ea = np.asarray(edge_attr, np.float32)

    ea_sum = np.zeros((N, 2), np.float32)
    np.add.at(ea_sum, dst, ea)
    cnt = np.bincount(dst, minlength=N).astype(np.float32)
    loop_attr = ea_sum / np.maximum(cnt, 1.0)[:, None]

    fsrc = np.concatenate([src, np.arange(N, dtype=np.int64)])
    fdst = np.concatenate([dst, np.arange(N, dtype=np.int64)])
    ffea = np.concatenate([ea, loop_attr], axis=0)

    key = fdst // NPC * NBLK + (fdst % NPC) // BLK
    order = np.argsort(key, kind="stable")
    kb = key[order]
    bounds = np.searchsorted(kb, np.arange(C * NBLK + 1))
    max_edges = int(np.max(np.diff(bounds)))
    tpb = (max_edges + TILE - 1) // TILE
    epb = tpb * TILE
    ec = NBLK * epb

    shards = []
    for c in range(C):
        s_src = np.zeros(ec, np.int16)
        s_dstloc = np.zeros(ec, np.int16)
        s_fea = np.zeros((ec, 2), np.float32)
        s_valid = np.zeros(ec, bool)
        for b in range(NBLK):
            k = c * NBLK + b
            el = order[bounds[k]:bounds[k + 1]]
            o = b * epb
            n = len(el)
            s_src[o:o + n] = fsrc[el].astype(np.int16)
            s_dstloc[o:o + n] = (fdst[el] - c * NPC).astype(np.int16)
            s_fea[o:o + n] = ffea[el]
            s_valid[o:o + n] = True
        t_ids = np.arange(ec) // TILE
        rel = s_dstloc.astype(np.float32) - (t_ids // tpb) * BLK
        rel[~s_valid] = -1.0  # padding matches no indicator column
        dstrel_f = np.ascontiguousarray(rel.reshape(ec // TILE, TILE).T)
        shards.append(dict(
            src_pk=_pack_idxs(s_src),
            feaT=np.ascontiguousarray(s_fea.T).astype(NP_BF16),
            dstrel=dstrel_f.astype(np.float32),
            # broadcast layout: every partition row = per-edge dst-rel value
            dstrelB=np.tile(rel.astype(NP_BF16)[None, :], (128, 1)),
        ))
    return shards, tpb


# --------------------------------------------------------------- device build
def _build(tpb, nzb, dims=DIMS, compile=True):
    key = (tpb, nzb, tuple(sorted(dims.items())))
    if key in _BUILD_CACHE:
        return _BUILD_CACHE[key]
    N, NPC, BLK, NBLK = dims["N"], dims["NPC"], dims["BLK"], dims["NBLK"]
    nz_bf, nz_bl, nz_br, nz_bo = nzb
    epb = tpb * TILE
    ec = NBLK * epb

    nc = bacc.Bacc("TRN2", target_bir_lowering=False, debug=False, num_devices=C)
    d_xT = nc.dram_tensor("xT", [F_IN, NPC], bf16, kind="ExternalInput")
    d_feaT = nc.dram_tensor("feaT", [2, ec], bf16, kind="ExternalInput")
    d_srcpk = nc.dram_tensor("src_pk", [128, ec // 16], i16, kind="ExternalInput")
    d_dstrel = nc.dram_tensor("dstrel", [128, ec // TILE], f32, kind="ExternalInput")
    d_dstrelB = nc.dram_tensor("dstrelB", [128, ec], bf16, kind="ExternalInput")
    d_eye = nc.dram_tensor("eye", [128, 128], bf16, kind="ExternalInput")
    d_iorow = nc.dram_tensor("iorow", [128, BLK], bf16, kind="ExternalInput")
    d_iocol = nc.dram_tensor("iocol", [BLK, 1], f32, kind="ExternalInput")
    d_Wf = nc.dram_tensor("Wf", [F_IN, D], bf16, kind="ExternalInput")
    d_Wl = nc.dram_tensor("Wl", [D, HD], bf16, kind="ExternalInput")
    d_Wr = nc.dram_tensor("Wr", [D, HD], bf16, kind="ExternalInput")
    d_We = nc.dram_tensor("We", [2, HD], bf16, kind="ExternalInput")
    d_attb = nc.dram_tensor("att_b", [128, HD], bf16, kind="ExternalInput")
    d_bf = nc.dram_tensor("bf_col", [128, 1], f32, kind="ExternalInput")
    d_blb = nc.dram_tensor("bl_b", [128, HD], f32, kind="ExternalInput")
    d_brb = nc.dram_tensor("br_b", [128, HD], f32, kind="ExternalInput")
    d_bob = nc.dram_tensor("bo_b", [128, D], f32, kind="ExternalInput")
    d_out = nc.dram_tensor("hout", [NPC, D], f32, kind="ExternalOutput")

    with tile.TileContext(nc) as tc, ExitStack() as ex:
        cst = ex.enter_context(tc.tile_pool(name="cst", bufs=1))
        dram = ex.enter_context(tc.tile_pool(name="dram", bufs=1, space="DRAM"))
        ps512 = ex.enter_context(tc.tile_pool(name="ps512", bufs=2, space="PSUM"))
        psO = ex.enter_context(tc.tile_pool(name="psO", bufs=2, space="PSUM"))
        psD = ex.enter_context(tc.tile_pool(name="psD", bufs=1, space="PSUM"))
        psT = ex.enter_context(tc.tile_pool(name="psT", bufs=1, space="PSUM"))
        gb1 = ex.enter_context(tc.tile_pool(name="gb1", bufs=3))
        scr = ex.enter_context(tc.tile_pool(name="scr", bufs=3))
        blkp = ex.enter_context(tc.tile_pool(name="blkp", bufs=3))
        evp = ex.enter_context(tc.tile_pool(name="evp", bufs=3))
        feap = ex.enter_context(tc.tile_pool(name="feap", bufs=2))

        nc.gpsimd.load_library(mlp)

        def ld(dt, shape, dtype=bf16):
            t = cst.tile(shape, dtype, name=f"sb_{dt.name}")
            nc.sync.dma_start(t[:], dt[:])
            return t

        eye = ld(d_eye, [128, 128])
        iorow = ld(d_iorow, [128, BLK])
        iocol = ld(d_iocol, [BLK, 1], f32)
        Wf = ld(d_Wf, [F_IN, D])
        Wl = ld(d_Wl, [D, HD])
        Wr = ld(d_Wr, [D, HD])
        We = ld(d_We, [2, HD])
        attb = ld(d_attb, [128, HD])
        xT = ld(d_xT, [F_IN, NPC])
        srcpk = ld(d_srcpk, [128, ec // 16], i16)
        dstrel = ld(d_dstrel, [128, ec // TILE], f32)
        dstrelB = ld(d_dstrelB, [128, ec])
        bf = ld(d_bf, [128, 1], f32) if nz_bf else None
        blb = ld(d_blb, [128, HD], f32) if nz_bl else None
        brb = ld(d_brb, [128, HD], f32) if nz_br else None
        bob = ld(d_bob, [128, D], f32) if nz_bo else None

        alpha_c = cst.tile([128, 1], f32, name="alpha_c")
        nc.vector.memset(alpha_c[:], NEG)
        hT = cst.tile([128, NPC], bf16, name="hT")
        xr_all = cst.tile([BLK, NBLK, HD], bf16, name="xr_all")
        agins = [dram.tile([NPC, HD], bf16, name=f"agin{i}")
                 for i in range(N_LAYERS)]
        agouts = [dram.tile([N, HD], bf16, addr_space="Shared", name=f"agout{i}")
                  for i in range(N_LAYERS)]

        # ---- layer-0 features, feature-major: h0T = Wf.T @ xT (+ bf)
        CH = min(NPC, 500)
        assert NPC % CH == 0
        for j in range(NPC // CH):
            ps = ps512.tile([128, CH], f32, tag="ps512")
            nc.tensor.matmul(ps[:], Wf[:], xT[:, j * CH:(j + 1) * CH],
                             start=True, stop=True)
            dst = hT[:, j * CH:(j + 1) * CH]
            if nz_bf:
                nc.vector.tensor_scalar_add(dst, ps[:], bf[:])
            else:
                nc.scalar.copy(dst, ps[:])

        def node_mms(j, m, pool):
            # xl/xr of layer j for node block m (uses current hT state)
            lh = hT[:, m * BLK:(m + 1) * BLK]
            psl = pool.tile([BLK, HD], f32, tag=pool.name)
            nc.tensor.matmul(psl[:], lh, Wl[:], start=True, stop=True)
            xle = evp.tile([BLK, HD], bf16, tag="ev")
            if nz_bl:
                nc.vector.tensor_add(xle[:], psl[:], blb[:BLK, :])
            else:
                nc.scalar.copy(xle[:], psl[:])
            nc.sync.dma_start(agins[j][m * BLK:(m + 1) * BLK, :], xle[:])
            psr = pool.tile([BLK, HD], f32, tag=pool.name)
            nc.tensor.matmul(psr[:], lh, Wr[:], start=True, stop=True)
            if nz_br:
                nc.vector.tensor_add(xr_all[:, m, :], psr[:], brb[:BLK, :])
            else:
                nc.scalar.copy(xr_all[:, m, :], psr[:])

        def full_ag(j):
            nc.gpsimd.collective_compute(
                "AllGather", ALU.bypass,
                replica_groups=[list(range(C))],
                ins=[agins[j].opt()], outs=[agouts[j].opt()],
            )

        def logits_phase(L, b):
            # gathers, indicators, z assembly, leaky+att-dot+exp for block b.
            # Returns the per-block tiles the scatter phase needs.
            agout = agouts[L]
            e0 = b * epb
            GC = 4  # tiles per gather call (512 idxs: SWDGE ring limit)
            xlg = gb1.tile([128, tpb, HD], bf16, tag="xlg")
            for g0 in range(0, tpb, GC):
                g1 = min(g0 + GC, tpb)
                ne = (g1 - g0) * TILE
                c0 = (e0 + g0 * TILE) // 16
                nc.gpsimd.dma_gather(xlg[:, g0:g1, :], agout[:],
                                     srcpk[:, c0:c0 + ne // 16],
                                     ne, ne, HD)
            feaT = feap.tile([2, epb], bf16, tag="feaT")
            nc.sync.dma_start(feaT[:], d_feaT[:, e0:e0 + epb])
            # node-major indicator for the whole block: B[n,e]=1 iff dst(e)=n
            Bn = blkp.tile([BLK, tpb, TILE], bf16, tag="Bn")
            nc.vector.tensor_scalar(
                Bn[:].rearrange("p a b -> p (a b)"),
                dstrelB[:BLK, e0:e0 + epb], iocol[:], None, ALU.is_equal)
            lgb = blkp.tile([128, tpb, H], f32, tag="lgb")
            indb = blkp.tile([128, tpb, BLK], bf16, tag="indb")
            assert tpb % 2 == 0
            for t0 in range(0, tpb, 2):
                zp = ps512.tile([128, 2, HD], f32, tag="ps512")
                for dt_ in range(2):
                    t = t0 + dt_
                    nc.vector.tensor_scalar(
                        indb[:, t, :], iorow[:],
                        dstrel[:, b * tpb + t:b * tpb + t + 1], None,
                        ALU.is_equal)
                    nc.tensor.matmul(zp[:, dt_, :],
                                     feaT[:, t * TILE:(t + 1) * TILE],
                                     We[:], start=True, stop=False)
                    nc.tensor.matmul(zp[:, dt_, :], Bn[:, t, :],
                                     xr_all[:, b, :],
                                     start=False, stop=False)
                for dt_ in range(2):
                    nc.tensor.matmul(zp[:, dt_, :], eye[:],
                                     xlg[:, t0 + dt_, :],
                                     start=False, stop=True)
                lz = scr.tile([128, 2, HD], bf16, tag="lz")
                nc.scalar.activation(
                    lz[:].rearrange("p a b -> p (a b)"),
                    zp[:].rearrange("p a b -> p (a b)"), AF.Prelu,
                    alpha=alpha_c[:])
                y = scr.tile([128, 2, HD], bf16, tag="y")
                for dt_ in range(2):
                    nc.vector.tensor_mul(y[:, dt_, :], lz[:, dt_, :],
                                         attb[:])
                nc.vector.tensor_reduce(
                    lgb[:, t0:t0 + 2, :],
                    y[:].rearrange("p a (h d) -> p (a h) d", h=H),
                    axis=mybir.AxisListType.X, op=ALU.add)
            webf = blkp.tile([128, tpb, H], f32, tag="webf")
            nc.scalar.activation(webf[:], lgb[:], AF.Exp)
            web = blkp.tile([128, tpb, H], bf16, tag="web")
            nc.scalar.activation(web[:], lgb[:], AF.Exp)
            return dict(xlg=xlg, indb=indb, webf=webf, web=web)

        def scatter_phase(L, b, st):
            xlg, indb, webf, web = st["xlg"], st["indb"], st["webf"], st["web"]
            outp = psO.tile([BLK, HD], f32, tag="psO")
            denp = psD.tile([BLK, H], f32, tag="psD")
            for t in range(tpb):
                msg = scr.tile([128, HD], bf16, tag="msg")
                for hh in range(H):
                    if hh < 2:
                        nc.scalar.activation(
                            msg[:, hh * D:(hh + 1) * D],
                            xlg[:, t, hh * D:(hh + 1) * D],
                            AF.Copy, scale=webf[:, t, hh:hh + 1])
                    else:
                        nc.vector.tensor_scalar_mul(
                            msg[:, hh * D:(hh + 1) * D],
                            xlg[:, t, hh * D:(hh + 1) * D],
                            webf[:, t, hh:hh + 1])
                nc.tensor.matmul(outp[:], indb[:, t, :], msg[:],
                                 start=(t == 0), stop=(t == tpb - 1))
                nc.tensor.matmul(denp[:], indb[:, t, :], web[:, t, :],
                                 start=(t == 0), stop=(t == tpb - 1))
            invd = blkp.tile([BLK, H], f32, tag="invd")
            nc.vector.reciprocal(invd[:], denp[:])
            # fold the head-mean 1/H into the normalizer
            nc.vector.tensor_scalar_mul(invd[:], invd[:], 1.0 / H)
            o = blkp.tile([BLK, HD], bf16, tag="o")
            for hh in range(H):
                nc.scalar.activation(
                    o[:, hh * D:(hh + 1) * D],
                    outp[:, hh * D:(hh + 1) * D],
                    AF.Copy, scale=invd[:, hh:hh + 1])
            s01 = blkp.tile([BLK, D], bf16, tag="s01")
            nc.vector.tensor_add(s01[:], o[:, 0:D], o[:, D:2 * D])
            s23 = blkp.tile([BLK, D], bf16, tag="s23")
            nc.vector.tensor_add(s23[:], o[:, 2 * D:3 * D], o[:, 3 * D:4 * D])
            sm = blkp.tile([BLK, D], bf16, tag="sm")
            if nz_bo:
                nc.vector.tensor_add(sm[:], s01[:], s23[:])
                nc.vector.tensor_add(sm[:], sm[:], bob[:BLK, :])
            else:
                nc.vector.tensor_add(sm[:], s01[:], s23[:])
            if L == N_LAYERS - 1:
                hb = blkp.tile([BLK, D], f32, tag="hbf")
                nc.vector.scalar_tensor_tensor(
                    hb[:], sm[:], 0.01, sm[:], ALU.mult, ALU.max)
                nc.sync.dma_start(d_out[b * BLK:(b + 1) * BLK, :], hb[:])
            else:
                hb = blkp.tile([BLK, D], bf16, tag="hb")
                nc.vector.scalar_tensor_tensor(
                    hb[:], sm[:], 0.01, sm[:], ALU.mult, ALU.max)
                tp = psT.tile([128, BLK], bf16, tag="psT")
                nc.tensor.transpose(tp[:], hb[:], eye[:BLK, :BLK])
                nc.scalar.copy(hT[:, b * BLK:(b + 1) * BLK], tp[:])
                # next layer's node matmuls right away, so the AllGather
                # launches as soon as the last block's output lands
                node_mms(L + 1, b, psT)
                if b == NBLK - 1:
                    full_ag(L + 1)

        for L in range(N_LAYERS):
            if L == 0:
                for m in range(NBLK):
                    node_mms(0, m, ps512)
                full_ag(0)
            # one-block software pipeline: emit block b's logits phase
            # before block b-1's scatter phase so the in-order tensor queue
            # never head-of-line blocks on the exp/msg chain
            prev = None
            for b in range(NBLK):
                st = logits_phase(L, b)
                if prev is not None:
                    scatter_phase(L, b - 1, prev)
                prev = st
            scatter_phase(L, NBLK - 1, prev)

    if compile:
        nc.compile()
    _BUILD_CACHE[key] = nc
    return nc


# ------------------------------------------------------------------ in_maps
def make_in_maps(inputs, dims=DIMS):
    N, NPC, BLK = dims["N"], dims["NPC"], dims["BLK"]
    x = np.asarray(inputs["x"], np.float32)
    Wf = np.ascontiguousarray(np.asarray(inputs["Wf"], np.float32))
    bf = np.asarray(inputs["bf"], np.float32)
    Wl = np.ascontiguousarray(np.asarray(inputs["Wl"], np.float32))
    bl = np.asarray(inputs["bl"], np.float32)
    Wr = np.ascontiguousarray(np.asarray(inputs["Wr"], np.float32))
    br = np.asarray(inputs["br"], np.float32)
    We = np.ascontiguousarray(np.asarray(inputs["We"], np.float32))
    att = np.asarray(inputs["att"], np.float32)
    bias_out = np.asarray(inputs["bias_out"], np.float32)

    shards, tpb = _build_shards(inputs["edge_index"], inputs["edge_attr"], dims)
    nzb = (bool(bf.any()), bool(bl.any()), bool(br.any()), bool(bias_out.any()))

    common = dict(
        eye=np.eye(128, dtype=NP_BF16),
        iorow=np.tile(np.arange(BLK, dtype=NP_BF16), (128, 1)),
        iocol=np.arange(BLK, dtype=np.float32).reshape(BLK, 1),
        Wf=Wf.astype(NP_BF16), Wl=Wl.astype(NP_BF16),
        Wr=Wr.astype(NP_BF16), We=We.astype(NP_BF16),
        att_b=np.tile(att.reshape(1, HD), (128, 1)).astype(NP_BF16),
        bf_col=np.ascontiguousarray(bf.reshape(D, 1)),
        bl_b=np.tile(bl.reshape(1, HD), (128, 1)).astype(np.float32),
        br_b=np.tile(br.reshape(1, HD), (128, 1)).astype(np.float32),
        bo_b=np.tile(bias_out.reshape(1, D), (128, 1)).astype(np.float32),
    )
    in_maps = []
    for c in range(C):
        sh = shards[c]
        m = dict(common)
        m["xT"] = np.ascontiguousarray(x[c * NPC:(c + 1) * NPC].T).astype(NP_BF16)
        m["feaT"] = sh["feaT"]
        m["src_pk"] = sh["src_pk"]
        m["dstrel"] = sh["dstrel"]
        m["dstrelB"] = sh["dstrelB"]
        in_maps.append(m)
    return in_maps, tpb, nzb


# -------------------------------------------------------------- bench hooks
def build_for_inputs(inputs):
    in_maps, tpb, nzb = make_in_maps(inputs, DIMS)
    nc = _build(tpb, nzb, DIMS)
    return nc, in_maps


def assemble_output(outs, out_names):
    NPC = DIMS["NPC"]
    got = np.asarray(outs[out_names.index("hout")]).reshape(C, NPC, -1)
    return got.reshape(C * NPC, -1).astype(np.float32)


# -------------------------------------------------------------------- kernel
def kernel(**inputs):
    in_maps, tpb, nzb = make_in_maps(inputs, DIMS)
    nc = _build(tpb, nzb, DIMS)
    res = run_bass_kernel_spmd(nc, in_maps, list(range(C)))
    NPC = DIMS["NPC"]
    return np.concatenate([res.results[c]["hout"] for c in range(C)], axis=0)


if __name__ == "__main__":
    nc = _build(10, (False, False, False, False), DIMS, compile=False)
    n_inst = sum(len(f.blocks[0].instructions) for f in nc.m.functions)
    print("trace-only build OK")


# revision 19
# speedup vs baseline: 4.2032x; 1.0214x over previous
"""GATv2 (3-layer, 4-head) on 8 Trainium2 NeuronCores — Bass/Tile SPMD kernel.

Sharding: destination-node partition (graph parallel). Core c owns dst nodes
[c*NPC, (c+1)*NPC) in NBLK blocks of BLK. Edges (incl. mean-filled
self-loops) are bucketed by dst block; all cores run one shared SPMD
program over padded, per-core index data.

Compute dtype is bf16 (PSUM accumulation fp32). Per layer:
  1. sharded node matmuls xl = h@Wl (to DRAM for AllGather) and
     xr = h@Wr (kept in SBUF — dst rows are block-local)
  2. AllGather of the bf16 xl table (only collective)
  3. per dst-block: dma_gather xl[src] rows only; xr[dst] is assembled
     with a node-major one-hot indicator matmul from the SBUF xr block;
     z = ee + xl_g + xr_g accumulated in PSUM,
     leaky_relu via one scalar_tensor_tensor on DVE, att-dot + per-head
     reduce on DVE, exp on ScalarE (softmax max-shift dropped: alpha is
     shift-invariant, logits are O(1)),
     msg = w * xl_g via ScalarE activation-with-scale,
     unnormalized scatter out += A^T @ msg and denom += A^T @ w
     via edge-major one-hot indicator matmuls,
     then normalize via ScalarE activation-with-scale (folding 1/H),
     head-mean, bias, outer leaky_relu.
"""
import sys

sys.path.insert(0, "/opt/trn_rl_repo")
from contextlib import ExitStack

import numpy as np
import concourse.bacc as bacc
import concourse.mybir as mybir
import concourse.tile as tile
from concourse.bass_utils import run_bass_kernel_spmd
from concourse.library_config import mlp

f32 = mybir.dt.float32
bf16 = mybir.dt.bfloat16
i16 = mybir.dt.int16
ALU = mybir.AluOpType
AF = mybir.ActivationFunctionType
NP_BF16 = mybir.dt.np(bf16)

H = 4
D = 128
HD = H * D
F_IN = 128
NEG = 0.2
N_LAYERS = 3
C = 8
TILE = 128

# full-problem dims (overridable for small-scale sim tests)
DIMS = dict(N=20000, NPC=2500, BLK=125, NBLK=20)

_BUILD_CACHE = {}


# ----------------------------------------------------------------- host prep
def _pack_idxs(il):
    n = len(il)
    a = np.zeros((128, n // 16), np.int16)
    base = il.reshape(n // 16, 16).T
    for g in range(8):
        a[g * 16:(g + 1) * 16] = base
    return a


def _build_shards(edge_index, edge_attr, dims=DIMS):
    N, NPC, BLK, NBLK = dims["N"], dims["NPC"], dims["BLK"], dims["NBLK"]
    src = np.asarray(edge_index[0], np.int64)
    dst = np.asarray(edge_index[1], np.int64)
    ea = np.asarray(edge_attr, np.float32)

    ea_sum = np.zeros((N, 2), np.float32)
    np.add.at(ea_sum, dst, ea)
    cnt = np.bincount(dst, minlength=N).astype(np.float32)
    loop_attr = ea_sum / np.maximum(cnt, 1.0)[:, None]

    fsrc = np.concatenate([src, np.arange(N, dtype=np.int64)])
    fdst = np.concatenate([dst, np.arange(N, dtype=np.int64)])
    ffea = np.concatenate([ea, loop_attr], axis=0)

    key = fdst // NPC * NBLK + (fdst % NPC) // BLK
    order = np.argsort(key, kind="stable")
    kb = key[order]
    bounds = np.searchsorted(kb, np.arange(C * NBLK + 1))
    max_edges = int(np.max(np.diff(bounds)))
    tpb = (max_edges + TILE - 1) // TILE
    epb = tpb * TILE
    ec = NBLK * epb

    shards = []
    for c in range(C):
        s_src = np.zeros(ec, np.int16)
        s_dstloc = np.zeros(ec, np.int16)
        s_fea = np.zeros((ec, 2), np.float32)
        s_valid = np.zeros(ec, bool)
        for b in range(NBLK):
            k = c * NBLK + b
            el = order[bounds[k]:bounds[k + 1]]
            o = b * epb
            n = len(el)
            s_src[o:o + n] = fsrc[el].astype(np.int16)
            s_dstloc[o:o + n] = (fdst[el] - c * NPC).astype(np.int16)
            s_fea[o:o + n] = ffea[el]
            s_valid[o:o + n] = True
        t_ids = np.arange(ec) // TILE
        rel = (s_dstloc.astype(np.int64) - (t_ids // tpb) * BLK)
        rel[~s_valid] = -1  # padding matches no indicator column
        # bnf: node-major one-hot indicator (rows 0..BLK-1) with the edge
        # features stacked as two extra contraction rows (125, 126)
        bnf = np.zeros((128, ec), NP_BF16)
        val = rel >= 0
        bnf[rel[val], np.nonzero(val)[0]] = 1.0
        bnf[BLK:BLK + 2, :] = s_fea.T
        # inda: edge-major one-hot indicator per tile [128, n_tiles*BLK]
        ntile = ec // TILE
        inda = np.zeros((128, ntile * BLK), NP_BF16)
        ep = np.arange(ec) % TILE
        tid = np.arange(ec) // TILE
        inda[ep[val], tid[val] * BLK + rel[val]] = 1.0
        shards.append(dict(
            src_pk=_pack_idxs(s_src),
            bnf=bnf,
            inda=inda,
        ))
    return shards, tpb


# --------------------------------------------------------------- device build
def _build(tpb, nzb, dims=DIMS, compile=True):
    key = (tpb, nzb, tuple(sorted(dims.items())))
    if key in _BUILD_CACHE:
        return _BUILD_CACHE[key]
    N, NPC, BLK, NBLK = dims["N"], dims["NPC"], dims["BLK"], dims["NBLK"]
    nz_bf, nz_bl, nz_br, nz_bo = nzb
    epb = tpb * TILE
    ec = NBLK * epb

    nc = bacc.Bacc("TRN2", target_bir_lowering=False, debug=False, num_devices=C)
    d_xT = nc.dram_tensor("xT", [F_IN, NPC], bf16, kind="ExternalInput")
    d_srcpk = nc.dram_tensor("src_pk", [128, ec // 16], i16, kind="ExternalInput")
    d_bnf = nc.dram_tensor("bnf", [128, ec], bf16, kind="ExternalInput")
    d_inda = nc.dram_tensor("inda", [128, (ec // TILE) * BLK], bf16,
                            kind="ExternalInput")
    d_eye = nc.dram_tensor("eye", [128, 128], bf16, kind="ExternalInput")
    d_Wf = nc.dram_tensor("Wf", [F_IN, D], bf16, kind="ExternalInput")
    d_Wl = nc.dram_tensor("Wl", [D, HD], bf16, kind="ExternalInput")
    d_Wr = nc.dram_tensor("Wr", [D, HD], bf16, kind="ExternalInput")
    d_We = nc.dram_tensor("We", [2, HD], bf16, kind="ExternalInput")
    d_attb = nc.dram_tensor("att_b", [128, HD], bf16, kind="ExternalInput")
    d_bf = nc.dram_tensor("bf_col", [128, 1], f32, kind="ExternalInput")
    d_blb = nc.dram_tensor("bl_b", [128, HD], f32, kind="ExternalInput")
    d_brb = nc.dram_tensor("br_b", [128, HD], f32, kind="ExternalInput")
    d_bob = nc.dram_tensor("bo_b", [128, D], f32, kind="ExternalInput")
    d_out = nc.dram_tensor("hout", [NPC, D], f32, kind="ExternalOutput")

    with tile.TileContext(nc) as tc, ExitStack() as ex:
        cst = ex.enter_context(tc.tile_pool(name="cst", bufs=1))
        dram = ex.enter_context(tc.tile_pool(name="dram", bufs=1, space="DRAM"))
        ps512 = ex.enter_context(tc.tile_pool(name="ps512", bufs=2, space="PSUM"))
        psO = ex.enter_context(tc.tile_pool(name="psO", bufs=2, space="PSUM"))
        psD = ex.enter_context(tc.tile_pool(name="psD", bufs=1, space="PSUM"))
        psT = ex.enter_context(tc.tile_pool(name="psT", bufs=1, space="PSUM"))
        gb1 = ex.enter_context(tc.tile_pool(name="gb1", bufs=3))
        scr = ex.enter_context(tc.tile_pool(name="scr", bufs=3))
        blkp = ex.enter_context(tc.tile_pool(name="blkp", bufs=3))
        evp = ex.enter_context(tc.tile_pool(name="evp", bufs=3))

        nc.gpsimd.load_library(mlp)

        def ld(dt, shape, dtype=bf16):
            t = cst.tile(shape, dtype, name=f"sb_{dt.name}")
            nc.sync.dma_start(t[:], dt[:])
            return t

        eye = ld(d_eye, [128, 128])
        Wf = ld(d_Wf, [F_IN, D])
        Wl = ld(d_Wl, [D, HD])
        Wr = ld(d_Wr, [D, HD])
        We = ld(d_We, [2, HD])
        attb = ld(d_attb, [128, HD])
        xT = ld(d_xT, [F_IN, NPC])
        srcpk = ld(d_srcpk, [128, ec // 16], i16)
        bf = ld(d_bf, [128, 1], f32) if nz_bf else None
        blb = ld(d_blb, [128, HD], f32) if nz_bl else None
        brb = ld(d_brb, [128, HD], f32) if nz_br else None
        bob = ld(d_bob, [128, D], f32) if nz_bo else None

        alpha_c = cst.tile([128, 1], f32, name="alpha_c")
        nc.vector.memset(alpha_c[:], NEG)
        hT = cst.tile([128, NPC], bf16, name="hT")
        xr_all = cst.tile([BLK + 2, NBLK, HD], bf16, name="xr_all")
        for m in range(NBLK):
            nc.sync.dma_start(xr_all[BLK:BLK + 2, m, :], d_We[:])
        agins = [dram.tile([NPC, HD], bf16, name=f"agin{i}")
                 for i in range(N_LAYERS)]
        agouts = [dram.tile([N, HD], bf16, addr_space="Shared", name=f"agout{i}")
                  for i in range(N_LAYERS)]

        # ---- layer-0 features, feature-major: h0T = Wf.T @ xT (+ bf)
        CH = min(NPC, 500)
        assert NPC % CH == 0
        for j in range(NPC // CH):
            ps = ps512.tile([128, CH], f32, tag="ps512")
            nc.tensor.matmul(ps[:], Wf[:], xT[:, j * CH:(j + 1) * CH],
                             start=True, stop=True)
            dst = hT[:, j * CH:(j + 1) * CH]
            if nz_bf:
                nc.vector.tensor_scalar_add(dst, ps[:], bf[:])
            else:
                nc.scalar.copy(dst, ps[:])

        def node_mms(j, m, pool):
            # xl/xr of layer j for node block m (uses current hT state)
            lh = hT[:, m * BLK:(m + 1) * BLK]
            psl = pool.tile([BLK, HD], f32, tag=pool.name)
            nc.tensor.matmul(psl[:], lh, Wl[:], start=True, stop=True)
            xle = evp.tile([BLK, HD], bf16, tag="ev")
            if nz_bl:
                nc.vector.tensor_add(xle[:], psl[:], blb[:BLK, :])
            else:
                nc.scalar.copy(xle[:], psl[:])
            nc.sync.dma_start(agins[j][m * BLK:(m + 1) * BLK, :], xle[:])
            psr = pool.tile([BLK, HD], f32, tag=pool.name)
            nc.tensor.matmul(psr[:], lh, Wr[:], start=True, stop=True)
            if nz_br:
                nc.vector.tensor_add(xr_all[:BLK, m, :], psr[:], brb[:BLK, :])
            else:
                nc.scalar.copy(xr_all[:BLK, m, :], psr[:])

        def full_ag(j):
            nc.gpsimd.collective_compute(
                "AllGather", ALU.bypass,
                replica_groups=[list(range(C))],
                ins=[agins[j].opt()], outs=[agouts[j].opt()],
            )

        def logits_phase(L, b):
            # gathers, indicators, z assembly, leaky+att-dot+exp for block b.
            # Returns the per-block tiles the scatter phase needs.
            agout = agouts[L]
            e0 = b * epb
            GC = 4  # tiles per gather call (512 idxs: SWDGE ring limit)
            xlg = gb1.tile([128, tpb, HD], bf16, tag="xlg")
            for g0 in range(0, tpb, GC):
                g1 = min(g0 + GC, tpb)
                ne = (g1 - g0) * TILE
                c0 = (e0 + g0 * TILE) // 16
                nc.gpsimd.dma_gather(xlg[:, g0:g1, :], agout[:],
                                     srcpk[:, c0:c0 + ne // 16],
                                     ne, ne, HD)
            # host-precomputed indicators: node-major one-hot with edge
            # features stacked as rows 125-126 (bnf), edge-major one-hot (inda)
            Bn = blkp.tile([BLK + 2, tpb, TILE], bf16, tag="Bn")
            nc.sync.dma_start(Bn[:].rearrange("p a b -> p (a b)"),
                              d_bnf[:BLK + 2, e0:e0 + epb])
            indb = blkp.tile([128, tpb, BLK], bf16, tag="indb")
            nc.sync.dma_start(
                indb[:].rearrange("p a b -> p (a b)"),
                d_inda[:, b * tpb * BLK:(b + 1) * tpb * BLK])
            lgb = blkp.tile([128, tpb, H], f32, tag="lgb")
            assert tpb % 2 == 0
            for t0 in range(0, tpb, 2):
                zp = ps512.tile([128, 2, HD], f32, tag="ps512")
                for dt_ in range(2):
                    t = t0 + dt_
                    nc.tensor.matmul(zp[:, dt_, :], Bn[:, t, :],
                                     xr_all[:, b, :],
                                     start=True, stop=False)
                for dt_ in range(2):
                    nc.tensor.matmul(zp[:, dt_, :], eye[:],
                                     xlg[:, t0 + dt_, :],
                                     start=False, stop=True)
                lz = scr.tile([128, 2, HD], bf16, tag="lz")
                nc.scalar.activation(
                    lz[:].rearrange("p a b -> p (a b)"),
                    zp[:].rearrange("p a b -> p (a b)"), AF.Prelu,
                    alpha=alpha_c[:])
                y = scr.tile([128, 2, HD], bf16, tag="y")
                for dt_ in range(2):
                    nc.vector.tensor_mul(y[:, dt_, :], lz[:, dt_, :],
                                         attb[:])
                nc.vector.tensor_reduce(
                    lgb[:, t0:t0 + 2, :],
                    y[:].rearrange("p a (h d) -> p (a h) d", h=H),
                    axis=mybir.AxisListType.X, op=ALU.add)
            webf = blkp.tile([128, tpb, H], f32, tag="webf")
            nc.scalar.activation(webf[:], lgb[:], AF.Exp)
            web = blkp.tile([128, tpb, H], bf16, tag="web")
            nc.scalar.activation(web[:], lgb[:], AF.Exp)
            return dict(xlg=xlg, indb=indb, webf=webf, web=web)

        def scatter_phase(L, b, st):
            xlg, indb, webf, web = st["xlg"], st["indb"], st["webf"], st["web"]
            outp = psO.tile([BLK, HD], f32, tag="psO")
            denp = psD.tile([BLK, H], f32, tag="psD")
            for t in range(tpb):
                msg = scr.tile([128, HD], bf16, tag="msg")
                for hh in range(H):
                    if hh < 2:
                        nc.scalar.activation(
                            msg[:, hh * D:(hh + 1) * D],
                            xlg[:, t, hh * D:(hh + 1) * D],
                            AF.Copy, scale=webf[:, t, hh:hh + 1])
                    else:
                        nc.vector.tensor_scalar_mul(
                            msg[:, hh * D:(hh + 1) * D],
                            xlg[:, t, hh * D:(hh + 1) * D],
                            webf[:, t, hh:hh + 1])
                nc.tensor.matmul(outp[:], indb[:, t, :], msg[:],
                                 start=(t == 0), stop=(t == tpb - 1))
                nc.tensor.matmul(denp[:], indb[:, t, :], web[:, t, :],
                                 start=(t == 0), stop=(t == tpb - 1))
            invd = blkp.tile([BLK, H], f32, tag="invd")
            nc.vector.reciprocal(invd[:], denp[:])
            # fold the head-mean 1/H into the normalizer
            nc.vector.tensor_scalar_mul(invd[:], invd[:], 1.0 / H)
            o = blkp.tile([BLK, HD], bf16, tag="o")
            for hh in range(H):
                nc.scalar.activation(
                    o[:, hh * D:(hh + 1) * D],
                    outp[:, hh * D:(hh + 1) * D],
                    AF.Copy, scale=invd[:, hh:hh + 1])
            sm = blkp.tile([BLK, D], bf16, tag="sm")
            with nc.allow_low_precision(reason="4-term head-mean in bf16"):
                nc.vector.tensor_reduce(
                    sm[:], o[:].rearrange("p (h d) -> p d h", h=H),
                    axis=mybir.AxisListType.X, op=ALU.add)
            if nz_bo:
                nc.vector.tensor_add(sm[:], sm[:], bob[:BLK, :])
            if L == N_LAYERS - 1:
                hb = blkp.tile([BLK, D], f32, tag="hbf")
                nc.vector.scalar_tensor_tensor(
                    hb[:], sm[:], 0.01, sm[:], ALU.mult, ALU.max)
                nc.sync.dma_start(d_out[b * BLK:(b + 1) * BLK, :], hb[:])
            else:
                hb = blkp.tile([BLK, D], bf16, tag="hb")
                nc.vector.scalar_tensor_tensor(
                    hb[:], sm[:], 0.01, sm[:], ALU.mult, ALU.max)
                tp = psT.tile([128, BLK], bf16, tag="psT")
                nc.tensor.transpose(tp[:], hb[:], eye[:BLK, :BLK])
                nc.scalar.copy(hT[:, b * BLK:(b + 1) * BLK], tp[:])
                # next layer's node matmuls right away, so the AllGather
                # launches as soon as the last block's output lands
                node_mms(L + 1, b, psT)
                if b == NBLK - 1:
                    full_ag(L + 1)

        for L in range(N_LAYERS):
            if L == 0:
                for m in range(NBLK):
                    node_mms(0, m, ps512)
                full_ag(0)
            # one-block software pipeline: emit block b's logits phase
            # before block b-1's scatter phase so the in-order tensor queue
            # never head-of-line blocks on the exp/msg chain
            prev = None
            for b in range(NBLK):
                st = logits_phase(L, b)
                if prev is not None:
                    scatter_phase(L, b - 1, prev)
                prev = st
            scatter_phase(L, NBLK - 1, prev)

    if compile:
        nc.compile()
    _BUILD_CACHE[key] = nc
    return nc


# ------------------------------------------------------------------ in_maps
def make_in_maps(inputs, dims=DIMS):
    N, NPC, BLK = dims["N"], dims["NPC"], dims["BLK"]
    x = np.asarray(inputs["x"], np.float32)
    Wf = np.ascontiguousarray(np.asarray(inputs["Wf"], np.float32))
    bf = np.asarray(inputs["bf"], np.float32)
    Wl = np.ascontiguousarray(np.asarray(inputs["Wl"], np.float32))
    bl = np.asarray(inputs["bl"], np.float32)
    Wr = np.ascontiguousarray(np.asarray(inputs["Wr"], np.float32))
    br = np.asarray(inputs["br"], np.float32)
    We = np.ascontiguousarray(np.asarray(inputs["We"], np.float32))
    att = np.asarray(inputs["att"], np.float32)
    bias_out = np.asarray(inputs["bias_out"], np.float32)

    shards, tpb = _build_shards(inputs["edge_index"], inputs["edge_attr"], dims)
    nzb = (bool(bf.any()), bool(bl.any()), bool(br.any()), bool(bias_out.any()))

    common = dict(
        eye=np.eye(128, dtype=NP_BF16),
        Wf=Wf.astype(NP_BF16), Wl=Wl.astype(NP_BF16),
        Wr=Wr.astype(NP_BF16), We=We.astype(NP_BF16),
        att_b=np.tile(att.reshape(1, HD), (128, 1)).astype(NP_BF16),
        bf_col=np.ascontiguousarray(bf.reshape(D, 1)),
        bl_b=np.tile(bl.reshape(1, HD), (128, 1)).astype(np.float32),
        br_b=np.tile(br.reshape(1, HD), (128, 1)).astype(np.float32),
        bo_b=np.tile(bias_out.reshape(1, D), (128, 1)).astype(np.float32),
    )
    in_maps = []
    for c in range(C):
        sh = shards[c]
        m = dict(common)
        m["xT"] = np.ascontiguousarray(x[c * NPC:(c + 1) * NPC].T).astype(NP_BF16)
        m["src_pk"] = sh["src_pk"]
        m["bnf"] = sh["bnf"]
        m["inda"] = sh["inda"]
        in_maps.append(m)
    return in_maps, tpb, nzb


# -------------------------------------------------------------- bench hooks
def build_for_inputs(inputs):
    in_maps, tpb, nzb = make_in_maps(inputs, DIMS)
    nc = _build(tpb, nzb, DIMS)
    return nc, in_maps


def assemble_output(outs, out_names):
    NPC = DIMS["NPC"]
    got = np.asarray(outs[out_names.index("hout")]).reshape(C, NPC, -1)
    return got.reshape(C * NPC, -1).astype(np.float32)


# -------------------------------------------------------------------- kernel
def kernel(**inputs):
    in_maps, tpb, nzb = make_in_maps(inputs, DIMS)
    nc = _build(tpb, nzb, DIMS)
    res = run_bass_kernel_spmd(nc, in_maps, list(range(C)))
    NPC = DIMS["NPC"]
    return np.concatenate([res.results[c]["hout"] for c in range(C)], axis=0)


if __name__ == "__main__":
    nc = _build(10, (False, False, False, False), DIMS, compile=False)
    n_inst = sum(len(f.blocks[0].instructions) for f in nc.m.functions)
    print("trace-only build OK")
